# revision 9
# baseline (speedup 1.0000x reference)
"""Swin-style windowed attention on 8 TRN2 NeuronCores.

Data-parallel over windows: core i handles windows [64i, 64i+64).
Per-window device pipeline (S^T layout, m on partitions):
  qk-proj -> PSUM -> SBUF (qT/kT head-aligned)
  v-proj (x as stationary, ones-augmented weight cols) -> v_aug [98, 33*6]
  S^T = k^T.T @ qT per (head, m-tile) into 6 PSUM banks
  E = exp(S^T)               (one wide ACT instr across banks)
  P = E * exp(biasT+maskT)   (host-precomputed table, one wide DVE mul)
  O^T_aug = v_aug.T @ P      (per head+m-tile, accumulated; row 32 = softmax sum)
  r = 1/s ; broadcast over partitions via DRAM bounce ; Z = O * r
  y = Z^T.T @ w_proj.T per n-tile -> DMA out (f32)
Host: folds softmax scale into w_qkv, gathers rel-pos bias, builds EBM table,
adds b_proj at the end.
"""

import numpy as np
import ml_dtypes

import concourse.bass as bass
import concourse.mybir as mybir
import concourse.tile as tile
from concourse import bacc
from concourse import bass_utils
from concourse.bass import AP

BF16 = mybir.dt.bfloat16
F32 = mybir.dt.float32
NPBF16 = ml_dtypes.bfloat16

B, N, C, H, HD, NG = 512, 196, 192, 6, 32, 64
NCORES = 8
WPC = B // NCORES  # 64 windows per core
MT = 98            # m-tile size, 2 tiles cover N=196

_CACHE = {}


def _build_nc():
    nc = bacc.Bacc("TRN2", target_bir_lowering=False, debug=False,
                   enable_asserts=False)

    xa_d = nc.dram_tensor("xa", [WPC, 97, 392], BF16, kind="ExternalInput").ap()
    ebm_d = nc.dram_tensor("ebm", [WPC, 98, 6 * 392], BF16, kind="ExternalInput").ap()
    wqk_d = nc.dram_tensor("wqk", [2, 96, 384], BF16, kind="ExternalInput").ap()
    wv_d = nc.dram_tensor("wv", [2, 97, 198], BF16, kind="ExternalInput").ap()
    wp_d = nc.dram_tensor("wp", [32, 6 * 192], BF16, kind="ExternalInput").ap()
    out_d = nc.dram_tensor("out", [WPC, N, C], F32, kind="ExternalOutput").ap()

    with tile.TileContext(nc) as tc:
        with (
            tc.tile_pool(name="static", bufs=1) as static_pool,
            tc.tile_pool(name="xa", bufs=2) as xa_pool,
            tc.tile_pool(name="ebm", bufs=2) as ebm_pool,
            tc.tile_pool(name="qk", bufs=2) as qk_pool,
            tc.tile_pool(name="vaug", bufs=2) as vaug_pool,
            tc.tile_pool(name="ep", bufs=2) as ep_pool,
            tc.tile_pool(name="zt", bufs=2) as zt_pool,
            tc.tile_pool(name="rr", bufs=2) as rr_pool,
            tc.tile_pool(name="spsum", bufs=1, space="PSUM") as s_psum,
            tc.tile_pool(name="ppsum", bufs=2, space="PSUM") as p_psum,
            tc.tile_pool(name="dram", bufs=2, space="DRAM") as dram_pool,
        ):
            # static weights
            wqk_t = []
            for kt in range(2):
                t = static_pool.tile([96, 384], BF16, tag=f"wqk{kt}")
                nc.sync.dma_start(t[:, :], wqk_d[kt])
                wqk_t.append(t)
            wv_t = []
            for kt in range(2):
                t = static_pool.tile([97, 198], BF16, tag=f"wv{kt}")
                nc.sync.dma_start(t[:, :], wv_d[kt])
                wv_t.append(t)
            wp_t = static_pool.tile([32, 6 * 192], BF16, tag="wp")
            nc.sync.dma_start(wp_t[:, :], wp_d[:, :])

            for w in range(WPC):
                # ---- input DMAs ----
                xa = xa_pool.tile([97, 392], BF16, tag="xa")
                nc.sync.dma_start(xa[:, :], xa_d[w])
                ebm = ebm_pool.tile([98, 6 * 392], BF16, tag="ebm")
                nc.sync.dma_start(ebm[:, :], ebm_d[w])

                # ---- qk projection: 3 feature tiles of 128 ----
                # feat f in [0,384): f<192 -> q head f//32, else k head
                qkps = []
                for ft in range(3):
                    ps = p_psum.tile([128, 512], F32, tag="ps2")
                    for kt in range(2):
                        nc.tensor.matmul(
                            ps[:, 0:196],
                            wqk_t[kt][:, ft * 128:(ft + 1) * 128],
                            xa[0:96, kt * 196:(kt + 1) * 196],
                            start=(kt == 0), stop=(kt == 1),
                        )
                    qkps.append(ps)

                # heads 0-2 in A tiles (rows 0-95), heads 3-5 in B tiles
                qT_A = qk_pool.tile([96, 196], BF16, tag="qTA")
                qT_B = qk_pool.tile([96, 196], BF16, tag="qTB")
                kT_A = qk_pool.tile([96, 196], BF16, tag="kTA")
                kT_B = qk_pool.tile([96, 196], BF16, tag="kTB")
                nc.vector.tensor_copy(qT_A[:, :], qkps[0][0:96, 0:196])
                nc.scalar.copy(qT_B[0:32, :], qkps[0][96:128, 0:196])
                nc.scalar.copy(qT_B[32:64, :], qkps[1][0:32, 0:196])
                nc.vector.tensor_copy(qT_B[64:96, :], qkps[1][32:64, 0:196])
                nc.vector.tensor_copy(kT_A[0:64, :], qkps[1][64:128, 0:196])
                nc.scalar.copy(kT_A[64:96, :], qkps[2][0:32, 0:196])
                nc.scalar.copy(kT_B[0:32, :], qkps[2][32:64, 0:196])
                nc.vector.tensor_copy(kT_B[32:64, :], qkps[2][64:96, 0:196])
                nc.scalar.copy(kT_B[64:96, :], qkps[2][96:128, 0:196])

                # ---- v projection (x stationary, ones-augmented) ----
                vaug = []
                for mt in range(2):
                    ps = p_psum.tile([128, 512], F32, tag="ps2")
                    for kt in range(2):
                        nc.tensor.matmul(
                            ps[0:98, 0:198],
                            xa[0:97, kt * 196 + mt * 98: kt * 196 + mt * 98 + 98],
                            wv_t[kt][:, :],
                            start=(kt == 0), stop=(kt == 1),
                        )
                    va = vaug_pool.tile([98, 198], BF16, tag="vaug")
                    nc.vector.tensor_copy(va[:, :], ps[0:98, 0:198])
                    vaug.append(va)

                # ---- QK^T -> S^T in 6 psum banks ----
                S = s_psum.tile([98, 6 * 512], F32, tag="S")
                for h in range(H):
                    if h < 3:
                        kTh, qTh, off = kT_A, qT_A, 32 * h
                    else:
                        kTh, qTh, off = kT_B, qT_B, 32 * (h - 3)
                    for mt in range(2):
                        nc.tensor.matmul(
                            S[:, h * 512 + mt * 196: h * 512 + (mt + 1) * 196],
                            kTh[off:off + 32, mt * 98: mt * 98 + 98],
                            qTh[off:off + 32, :],
                            start=True, stop=True,
                        )

                # ---- exp (one wide instr across banks) ----
                E = ep_pool.tile([98, 6 * 392], BF16, tag="E")
                S3 = S[:, :].rearrange("p (h x) -> p h x", h=6)[:, :, 0:392]
                E3 = E[:, :].rearrange("p (h x) -> p h x", h=6)
                nc.scalar.activation(E3, S3, mybir.ActivationFunctionType.Exp)

                # ---- P = E * EBM ----
                P = ep_pool.tile([98, 6 * 392], BF16, tag="P")
                nc.vector.tensor_mul(P[:, :], E[:, :], ebm[:, :])

                # ---- PV: O^T_aug per head, 4 heads in bank A, 2 in bank B ----
                # bank A: h0 rows0-32 free0:196, h1 rows64-96 free0:196,
                #         h2 rows0-32 free196:392, h3 rows64-96 free196:392
                O_A = p_psum.tile([128, 512], F32, tag="ps2")
                O_B = p_psum.tile([128, 512], F32, tag="ps2")
                for h in range(H):
                    if h < 4:
                        O, row, fo = O_A, 64 * (h % 2), 196 * (h // 2)
                    else:
                        O, row, fo = O_B, 64 * (h % 2), 0
                    for mt in range(2):
                        nc.tensor.matmul(
                            O[row:row + 33, fo:fo + 196],
                            vaug[mt][:, 33 * h: 33 * h + 33],
                            P[:, h * 392 + mt * 196: h * 392 + (mt + 1) * 196],
                            start=(mt == 0), stop=(mt == 1),
                        )

                # ---- reciprocal of softmax sums ----
                # s rows: partition 32 = [s0|s2] (A) / [s4] (B); 96 = odd heads
                rE = rr_pool.tile([1, 588], F32, tag="rE")
                rO = rr_pool.tile([1, 588], F32, tag="rO")
                nc.vector.reciprocal(rE[0:1, 0:392], O_A[32:33, 0:392])
                nc.vector.reciprocal(rE[0:1, 392:588], O_B[32:33, 0:196])
                nc.vector.reciprocal(rO[0:1, 0:392], O_A[96:97, 0:392])
                nc.vector.reciprocal(rO[0:1, 392:588], O_B[96:97, 0:196])

                # ---- broadcast r across 32 partitions via DRAM bounce ----
                scr = dram_pool.tile([2, 588], F32, tag="scr")
                nc.sync.dma_start(scr[0:1, :], rE[:, :])
                nc.sync.dma_start(scr[1:2, :], rO[:, :])
                R_e = rr_pool.tile([32, 588], F32, tag="Re")
                R_o = rr_pool.tile([32, 588], F32, tag="Ro")
                bc_e = AP(scr[:, :].tensor, scr[:, :].offset, [[0, 32], [1, 588]])
                bc_o = AP(scr[:, :].tensor, scr[1:2, :].offset, [[0, 32], [1, 588]])
                nc.sync.dma_start(R_e[:, :], bc_e)
                nc.sync.dma_start(R_o[:, :], bc_o)

                # ---- Z = O * r ---- (bf16 out, head layout (pair, n))
                ztAe = zt_pool.tile([32, 392], BF16, tag="ztAe")  # h0, h2
                ztAo = zt_pool.tile([32, 392], BF16, tag="ztAo")  # h1, h3
                ztBe = zt_pool.tile([32, 196], BF16, tag="ztBe")  # h4
                ztBo = zt_pool.tile([32, 196], BF16, tag="ztBo")  # h5
                nc.vector.tensor_mul(ztAe[:, :], O_A[0:32, 0:392], R_e[:, 0:392])
                nc.vector.tensor_mul(ztAo[:, :], O_A[64:96, 0:392], R_o[:, 0:392])
                nc.vector.tensor_mul(ztBe[:, :], O_B[0:32, 0:196], R_e[:, 392:588])
                nc.vector.tensor_mul(ztBo[:, :], O_B[64:96, 0:196], R_o[:, 392:588])

                # ---- projection: y[n_tile, c'] = sum_h Z_h^T.T @ wp_h ----
                zt_of = {0: (ztAe, 0), 2: (ztAe, 196), 1: (ztAo, 0),
                         3: (ztAo, 196), 4: (ztBe, 0), 5: (ztBo, 0)}
                for nt in range(2):
                    yp = p_psum.tile([128, 512], F32, tag="ps2")
                    for h in range(H):
                        zt_t, fo = zt_of[h]
                        nc.tensor.matmul(
                            yp[0:98, 0:192],
                            zt_t[:, fo + nt * 98: fo + nt * 98 + 98],
                            wp_t[:, h * 192:(h + 1) * 192],
                            start=(h == 0), stop=(h == 5),
                        )
                    y_sb = zt_pool.tile([98, 192], F32, tag="ysb")
                    if nt == 0:
                        nc.vector.tensor_copy(y_sb[:, :], yp[0:98, 0:192])
                    else:
                        nc.scalar.copy(y_sb[:, :], yp[0:98, 0:192])
                    nc.sync.dma_start(out_d[w, nt * 98:(nt + 1) * 98, :],
                                      y_sb[:, :])
    nc.compile()
    return nc


def _host_precompute(x, w_qkv, w_proj, bias_table, mask, rel_index):
    scale = HD ** (-0.5)
    wq = np.array(w_qkv, np.float32).copy()
    wq[0:C] *= scale  # fold softmax scale into q weights

    # xa[w, p, kt*196 + j] = x[w, j, kt*96 + p]; row 96: kt0->0, kt1->1
    xT = np.ascontiguousarray(np.transpose(np.asarray(x, np.float32), (0, 2, 1)))
    xa = np.zeros((B, 97, 392), np.float32)
    xa[:, 0:96, 0:196] = xT[:, 0:96]
    xa[:, 0:96, 196:392] = xT[:, 96:192]
    xa[:, 96, 196:392] = 1.0

    # wqk[kt, p, f] = wq[f, kt*96+p]  (f < 384: q then k features)
    wqkT = wq[0:384].T  # [192, 384]
    wqk = np.stack([wqkT[0:96], wqkT[96:192]])

    # wv[kt, p, 33h+d] = wq[384+32h+d, kt*96+p]; ones row kt1 p=96
    wv = np.zeros((2, 97, 198), np.float32)
    wvT = wq[384:576].T  # [192, 192] [c, (h,d)]
    for h in range(H):
        wv[0, 0:96, 33 * h: 33 * h + 32] = wvT[0:96, 32 * h: 32 * h + 32]
        wv[1, 0:96, 33 * h: 33 * h + 32] = wvT[96:192, 32 * h: 32 * h + 32]
        wv[1, 96, 33 * h + 32] = 1.0

    # wp[p, h*192 + c'] = w_proj[c', 32h + p]
    wp = np.zeros((32, 6 * 192), np.float32)
    wpT = np.asarray(w_proj, np.float32).T  # [c, c']
    for h in range(H):
        wp[:, h * 192:(h + 1) * 192] = wpT[32 * h: 32 * h + 32]

    # EBM[w, p, h*392 + mt*196 + n] = exp(bias[n, m, h] + mask[w, n, m]),
    # m = mt*98 + p
    bias = np.asarray(bias_table, np.float32)[np.asarray(rel_index).reshape(-1)]
    bias = bias.reshape(N, N, H)  # [n, m, h]
    biasT = np.transpose(bias, (2, 1, 0))  # [h, m, n]
    maskT = np.transpose(np.asarray(mask, np.float32), (0, 2, 1))  # [g, m, n]
    ebm = np.exp(biasT[None] + maskT[:, None])  # [g, h, m, n]
    ebm = ebm.reshape(NG, H, 2, MT, N).transpose(0, 3, 1, 2, 4)
    ebm = np.ascontiguousarray(ebm.reshape(NG, MT, H * 392))

    return (xa.astype(NPBF16), wqk.astype(NPBF16), wv.astype(NPBF16),
            wp.astype(NPBF16), ebm.astype(NPBF16))


def kernel(x, w_qkv, w_proj, b_proj, bias_table, mask, rel_index):
    xa, wqk, wv, wp, ebm = _host_precompute(
        x, w_qkv, w_proj, bias_table, mask, rel_index)

    if "nc" not in _CACHE:
        _CACHE["nc"] = _build_nc()
    nc = _CACHE["nc"]

    in_maps = []
    for c in range(NCORES):
        in_maps.append({
            "xa": np.ascontiguousarray(xa[c * WPC:(c + 1) * WPC]),
            "ebm": ebm,  # window w on core uses mask (64c+w) % 64 = w
            "wqk": wqk, "wv": wv, "wp": wp,
        })

    res = bass_utils.run_bass_kernel_spmd(nc, in_maps, core_ids=list(range(NCORES)))
    out = np.concatenate([res.results[c]["out"] for c in range(NCORES)], axis=0)
    out = out.astype(np.float32) + np.asarray(b_proj, np.float32)[None, None, :]
    return out


# revision 12
# speedup vs baseline: 1.0506x; 1.0506x over previous
"""Swin-style windowed attention on 8 TRN2 NeuronCores.

Data-parallel over windows: core i handles windows [64i, 64i+64).
Per-window device pipeline (S^T layout, m on partitions):
  qk-proj -> PSUM -> SBUF (qT/kT head-aligned)
  v-proj (x as stationary, ones-augmented weight cols) -> v_aug [98, 33*6]
  S^T = k^T.T @ qT per (head, m-tile) into 6 PSUM banks
  E = exp(S^T)               (one wide ACT instr across banks)
  P = E * exp(biasT+maskT)   (host-precomputed table, one wide DVE mul)
  O^T_aug = v_aug.T @ P      (per head+m-tile, accumulated; row 32 = softmax sum)
  r = 1/s ; broadcast over partitions via DRAM bounce ; Z = O * r
  y = Z^T.T @ w_proj.T per n-tile -> DMA out (f32)
Host: folds softmax scale into w_qkv, gathers rel-pos bias, builds EBM table,
adds b_proj at the end.
"""

import numpy as np
import ml_dtypes

import concourse.bass as bass
import concourse.mybir as mybir
import concourse.tile as tile
from concourse import bacc
from concourse import bass_utils
from concourse.bass import AP

BF16 = mybir.dt.bfloat16
F32 = mybir.dt.float32
NPBF16 = ml_dtypes.bfloat16

B, N, C, H, HD, NG = 512, 196, 192, 6, 32, 64
NCORES = 8
WPC = B // NCORES  # 64 windows per core
MT = 98            # m-tile size, 2 tiles cover N=196

_CACHE = {}


def _build_nc():
    nc = bacc.Bacc("TRN2", target_bir_lowering=False, debug=False,
                   enable_asserts=False)

    xa_d = nc.dram_tensor("xa", [WPC, 97, 392], BF16, kind="ExternalInput").ap()
    ebm_d = nc.dram_tensor("ebm", [WPC, 98, 6 * 392], BF16, kind="ExternalInput").ap()
    wqk_d = nc.dram_tensor("wqk", [2, 96, 384], BF16, kind="ExternalInput").ap()
    wv_d = nc.dram_tensor("wv", [2, 97, 198], BF16, kind="ExternalInput").ap()
    wp_d = nc.dram_tensor("wp", [32, 6 * 192], BF16, kind="ExternalInput").ap()
    out_d = nc.dram_tensor("out", [WPC, N, C], F32, kind="ExternalOutput").ap()

    with tile.TileContext(nc) as tc:
        with (
            tc.tile_pool(name="static", bufs=1) as static_pool,
            tc.tile_pool(name="xa", bufs=2) as xa_pool,
            tc.tile_pool(name="ebm", bufs=2) as ebm_pool,
            tc.tile_pool(name="qk", bufs=2) as qk_pool,
            tc.tile_pool(name="vaug", bufs=2) as vaug_pool,
            tc.tile_pool(name="ep", bufs=2) as ep_pool,
            tc.tile_pool(name="zt", bufs=2) as zt_pool,
            tc.tile_pool(name="rr", bufs=2) as rr_pool,
            tc.tile_pool(name="spsum", bufs=1, space="PSUM") as s_psum,
            tc.tile_pool(name="opsum", bufs=3, space="PSUM") as o_psum,
            tc.tile_pool(name="ppsum", bufs=2, space="PSUM") as p_psum,
            tc.tile_pool(name="dram", bufs=2, space="DRAM") as dram_pool,
        ):
            # static weights
            wqk_t = []
            for kt in range(2):
                t = static_pool.tile([96, 384], BF16, tag=f"wqk{kt}")
                nc.sync.dma_start(t[:, :], wqk_d[kt])
                wqk_t.append(t)
            wv_t = []
            for kt in range(2):
                t = static_pool.tile([97, 198], BF16, tag=f"wv{kt}")
                nc.sync.dma_start(t[:, :], wv_d[kt])
                wv_t.append(t)
            wp_t = static_pool.tile([32, 6 * 192], BF16, tag="wp")
            nc.sync.dma_start(wp_t[:, :], wp_d[:, :])

            for w in range(WPC):
                # ---- input DMAs ----
                xa = xa_pool.tile([97, 392], BF16, tag="xa")
                nc.sync.dma_start(xa[:, :], xa_d[w])
                ebm = ebm_pool.tile([98, 6 * 392], BF16, tag="ebm")
                nc.sync.dma_start(ebm[:, :], ebm_d[w])

                # ---- qk projection: 3 feature tiles of 128 ----
                # feat f in [0,384): f<192 -> q head f//32, else k head
                qkps = []
                for ft in range(3):
                    ps = p_psum.tile([128, 512], F32, tag="ps2")
                    for kt in range(2):
                        nc.tensor.matmul(
                            ps[:, 0:196],
                            wqk_t[kt][:, ft * 128:(ft + 1) * 128],
                            xa[0:96, kt * 196:(kt + 1) * 196],
                            start=(kt == 0), stop=(kt == 1),
                        )
                    qkps.append(ps)

                # heads 0-2 in A tiles (rows 0-95), heads 3-5 in B tiles
                qT_A = qk_pool.tile([96, 196], BF16, tag="qTA")
                qT_B = qk_pool.tile([96, 196], BF16, tag="qTB")
                kT_A = qk_pool.tile([96, 196], BF16, tag="kTA")
                kT_B = qk_pool.tile([96, 196], BF16, tag="kTB")
                nc.vector.tensor_copy(qT_A[:, :], qkps[0][0:96, 0:196])
                nc.scalar.copy(qT_B[0:32, :], qkps[0][96:128, 0:196])
                nc.scalar.copy(qT_B[32:64, :], qkps[1][0:32, 0:196])
                nc.vector.tensor_copy(qT_B[64:96, :], qkps[1][32:64, 0:196])
                nc.vector.tensor_copy(kT_A[0:64, :], qkps[1][64:128, 0:196])
                nc.scalar.copy(kT_A[64:96, :], qkps[2][0:32, 0:196])
                nc.scalar.copy(kT_B[0:32, :], qkps[2][32:64, 0:196])
                nc.vector.tensor_copy(kT_B[32:64, :], qkps[2][64:96, 0:196])
                nc.scalar.copy(kT_B[64:96, :], qkps[2][96:128, 0:196])

                # ---- v projection (x stationary, ones-augmented) ----
                vaug = []
                for mt in range(2):
                    ps = p_psum.tile([128, 512], F32, tag="ps2")
                    for kt in range(2):
                        nc.tensor.matmul(
                            ps[0:98, 0:198],
                            xa[0:97, kt * 196 + mt * 98: kt * 196 + mt * 98 + 98],
                            wv_t[kt][:, :],
                            start=(kt == 0), stop=(kt == 1),
                        )
                    va = vaug_pool.tile([98, 198], BF16, tag="vaug")
                    nc.vector.tensor_copy(va[:, :], ps[0:98, 0:198])
                    vaug.append(va)

                # ---- per half (3 heads): QK^T -> S^T -> exp -> mul -> PV ----
                # half 0 = heads 0-2 (A tiles), half 1 = heads 3-5 (B tiles)
                s_sb = rr_pool.tile([1, 1176], F32, tag="ssb")
                P = ep_pool.tile([98, 6 * 392], BF16, tag="P")
                O_halves = []
                for hf in range(2):
                    kTh, qTh = (kT_A, qT_A) if hf == 0 else (kT_B, qT_B)
                    S = s_psum.tile([98, 3 * 512], F32, tag="S")
                    for hl in range(3):
                        for mt in range(2):
                            nc.tensor.matmul(
                                S[:, hl * 512 + mt * 196: hl * 512 + (mt + 1) * 196],
                                kTh[32 * hl:32 * hl + 32, mt * 98: mt * 98 + 98],
                                qTh[32 * hl:32 * hl + 32, :],
                                start=True, stop=True,
                            )
                    # exp across the 3 banks in one instr
                    E = ep_pool.tile([98, 3 * 392], BF16, tag="E")
                    S3 = S[:, :].rearrange("p (h x) -> p h x", h=3)[:, :, 0:392]
                    E3 = E[:, :].rearrange("p (h x) -> p h x", h=3)
                    nc.scalar.activation(E3, S3, mybir.ActivationFunctionType.Exp)
                    nc.vector.tensor_mul(
                        P[:, hf * 1176:(hf + 1) * 1176], E[:, :],
                        ebm[:, hf * 1176:(hf + 1) * 1176])

                    # PV into one 1-bank tile:
                    # local0: rows0-32 free0:196; local1: rows64-96 free0:196;
                    # local2: rows0-32 free196:392
                    O = o_psum.tile([98, 512], F32, tag="O")
                    O_halves.append(O)
                    for hl in range(3):
                        h = 3 * hf + hl
                        row = 64 if hl == 1 else 0
                        fo = 196 if hl == 2 else 0
                        for mt in range(2):
                            nc.tensor.matmul(
                                O[row:row + 33, fo:fo + 196],
                                vaug[mt][:, 33 * h: 33 * h + 33],
                                P[:, h * 392 + mt * 196: h * 392 + (mt + 1) * 196],
                                start=(mt == 0), stop=(mt == 1),
                            )
                    # s rows -> sbuf staging: [s_l0|s_l2] at row32, s_l1 at 96
                    nc.vector.tensor_copy(s_sb[0:1, hf * 588: hf * 588 + 392],
                                          O[32:33, 0:392])
                    nc.vector.tensor_copy(s_sb[0:1, hf * 588 + 392:(hf + 1) * 588],
                                          O[96:97, 0:196])

                # ---- reciprocal via DRAM transpose bounce ----
                scr = dram_pool.tile([1, 1176], F32, tag="scr")
                nc.gpsimd.dma_start(scr[0:1, :], s_sb[:, :])
                s_t = rr_pool.tile([98, 12], F32, tag="st")
                tr = AP(scr[:, :].tensor, scr[:, :].offset, [[12, 98], [1, 12]])
                nc.gpsimd.dma_start(s_t[:, :], tr)
                r_t = rr_pool.tile([98, 12], F32, tag="rt")
                nc.vector.reciprocal(r_t[:, :], s_t[:, :])
                scr2 = dram_pool.tile([1, 1176], F32, tag="scr2")
                tr2 = AP(scr2[:, :].tensor, scr2[:, :].offset, [[12, 98], [1, 12]])
                nc.gpsimd.dma_start(tr2, r_t[:, :])
                R_all = rr_pool.tile([32, 1176], F32, tag="Rall")
                bc = AP(scr2[:, :].tensor, scr2[:, :].offset, [[0, 32], [1, 1176]])
                nc.gpsimd.dma_start(R_all[:, :], bc)

                # ---- Z = O * r ---- (bf16 out; layout per half as in O)
                ztA = zt_pool.tile([32, 392], BF16, tag="ztA")  # h0, h2
                ztAm = zt_pool.tile([32, 196], BF16, tag="ztAm")  # h1
                ztB = zt_pool.tile([32, 392], BF16, tag="ztB")  # h3, h5
                ztBm = zt_pool.tile([32, 196], BF16, tag="ztBm")  # h4
                nc.vector.tensor_mul(ztA[:, :], O_halves[0][0:32, 0:392],
                                     R_all[:, 0:392])
                nc.vector.tensor_mul(ztAm[:, :], O_halves[0][64:96, 0:196],
                                     R_all[:, 392:588])
                nc.vector.tensor_mul(ztB[:, :], O_halves[1][0:32, 0:392],
                                     R_all[:, 588:980])
                nc.vector.tensor_mul(ztBm[:, :], O_halves[1][64:96, 0:196],
                                     R_all[:, 980:1176])

                # ---- projection: y[n_tile, c'] = sum_h Z_h^T.T @ wp_h ----
                zt_of = {0: (ztA, 0), 2: (ztA, 196), 1: (ztAm, 0),
                         3: (ztB, 0), 5: (ztB, 196), 4: (ztBm, 0)}
                for nt in range(2):
                    yp = p_psum.tile([128, 512], F32, tag="ps2")
                    for h in range(H):
                        zt_t, fo = zt_of[h]
                        nc.tensor.matmul(
                            yp[0:98, 0:192],
                            zt_t[:, fo + nt * 98: fo + nt * 98 + 98],
                            wp_t[:, h * 192:(h + 1) * 192],
                            start=(h == 0), stop=(h == 5),
                        )
                    y_sb = zt_pool.tile([98, 192], F32, tag="ysb")
                    if nt == 0:
                        nc.vector.tensor_copy(y_sb[:, :], yp[0:98, 0:192])
                    else:
                        nc.scalar.copy(y_sb[:, :], yp[0:98, 0:192])
                    nc.sync.dma_start(out_d[w, nt * 98:(nt + 1) * 98, :],
                                      y_sb[:, :])
    nc.compile()
    return nc


def _host_precompute(x, w_qkv, w_proj, bias_table, mask, rel_index):
    scale = HD ** (-0.5)
    wq = np.array(w_qkv, np.float32).copy()
    wq[0:C] *= scale  # fold softmax scale into q weights

    # xa[w, p, kt*196 + j] = x[w, j, kt*96 + p]; row 96: kt0->0, kt1->1
    xT = np.ascontiguousarray(np.transpose(np.asarray(x, np.float32), (0, 2, 1)))
    xa = np.zeros((B, 97, 392), np.float32)
    xa[:, 0:96, 0:196] = xT[:, 0:96]
    xa[:, 0:96, 196:392] = xT[:, 96:192]
    xa[:, 96, 196:392] = 1.0

    # wqk[kt, p, f] = wq[f, kt*96+p]  (f < 384: q then k features)
    wqkT = wq[0:384].T  # [192, 384]
    wqk = np.stack([wqkT[0:96], wqkT[96:192]])

    # wv[kt, p, 33h+d] = wq[384+32h+d, kt*96+p]; ones row kt1 p=96
    wv = np.zeros((2, 97, 198), np.float32)
    wvT = wq[384:576].T  # [192, 192] [c, (h,d)]
    for h in range(H):
        wv[0, 0:96, 33 * h: 33 * h + 32] = wvT[0:96, 32 * h: 32 * h + 32]
        wv[1, 0:96, 33 * h: 33 * h + 32] = wvT[96:192, 32 * h: 32 * h + 32]
        wv[1, 96, 33 * h + 32] = 1.0

    # wp[p, h*192 + c'] = w_proj[c', 32h + p]
    wp = np.zeros((32, 6 * 192), np.float32)
    wpT = np.asarray(w_proj, np.float32).T  # [c, c']
    for h in range(H):
        wp[:, h * 192:(h + 1) * 192] = wpT[32 * h: 32 * h + 32]

    # EBM[w, p, h*392 + mt*196 + n] = exp(bias[n, m, h] + mask[w, n, m]),
    # m = mt*98 + p
    bias = np.asarray(bias_table, np.float32)[np.asarray(rel_index).reshape(-1)]
    bias = bias.reshape(N, N, H)  # [n, m, h]
    biasT = np.transpose(bias, (2, 1, 0))  # [h, m, n]
    maskT = np.transpose(np.asarray(mask, np.float32), (0, 2, 1))  # [g, m, n]
    ebm = np.exp(biasT[None] + maskT[:, None])  # [g, h, m, n]
    ebm = ebm.reshape(NG, H, 2, MT, N).transpose(0, 3, 1, 2, 4)
    ebm = np.ascontiguousarray(ebm.reshape(NG, MT, H * 392))

    return (xa.astype(NPBF16), wqk.astype(NPBF16), wv.astype(NPBF16),
            wp.astype(NPBF16), ebm.astype(NPBF16))


def kernel(x, w_qkv, w_proj, b_proj, bias_table, mask, rel_index):
    xa, wqk, wv, wp, ebm = _host_precompute(
        x, w_qkv, w_proj, bias_table, mask, rel_index)

    if "nc" not in _CACHE:
        _CACHE["nc"] = _build_nc()
    nc = _CACHE["nc"]

    in_maps = []
    for c in range(NCORES):
        in_maps.append({
            "xa": np.ascontiguousarray(xa[c * WPC:(c + 1) * WPC]),
            "ebm": ebm,  # window w on core uses mask (64c+w) % 64 = w
            "wqk": wqk, "wv": wv, "wp": wp,
        })

    res = bass_utils.run_bass_kernel_spmd(nc, in_maps, core_ids=list(range(NCORES)))
    out = np.concatenate([res.results[c]["out"] for c in range(NCORES)], axis=0)
    out = out.astype(np.float32) + np.asarray(b_proj, np.float32)[None, None, :]
    return out


# revision 13
# speedup vs baseline: 1.2224x; 1.1636x over previous
"""Swin-style windowed attention on 8 TRN2 NeuronCores.

Data-parallel over windows: core i handles windows [64i, 64i+64).
Per-window device pipeline (S^T layout, m on partitions):
  qk-proj -> PSUM -> SBUF (qT/kT head-aligned)
  v-proj (x as stationary, ones-augmented weight cols) -> v_aug [98, 33*6]
  S^T = k^T.T @ qT per (head, m-tile) into 6 PSUM banks
  E = exp(S^T)               (one wide ACT instr across banks)
  P = E * exp(biasT+maskT)   (host-precomputed table, one wide DVE mul)
  O^T_aug = v_aug.T @ P      (per head+m-tile, accumulated; row 32 = softmax sum)
  r = 1/s ; broadcast over partitions via DRAM bounce ; Z = O * r
  y = Z^T.T @ w_proj.T per n-tile -> DMA out (f32)
Host: folds softmax scale into w_qkv, gathers rel-pos bias, builds EBM table,
adds b_proj at the end.
"""

import numpy as np
import ml_dtypes

import concourse.bass as bass
import concourse.mybir as mybir
import concourse.tile as tile
from concourse import bacc
from concourse import bass_utils
from concourse.bass import AP

BF16 = mybir.dt.bfloat16
F32 = mybir.dt.float32
NPBF16 = ml_dtypes.bfloat16

B, N, C, H, HD, NG = 512, 196, 192, 6, 32, 64
NCORES = 8
WPC = B // NCORES  # 64 windows per core
MT = 98            # m-tile size, 2 tiles cover N=196

_CACHE = {}


def _build_nc():
    nc = bacc.Bacc("TRN2", target_bir_lowering=False, debug=False,
                   enable_asserts=False)

    xa_d = nc.dram_tensor("xa", [WPC, 97, 392], BF16, kind="ExternalInput").ap()
    ebm_d = nc.dram_tensor("ebm", [WPC, 98, 6 * 392], BF16, kind="ExternalInput").ap()
    wqk_d = nc.dram_tensor("wqk", [2, 96, 384], BF16, kind="ExternalInput").ap()
    wv_d = nc.dram_tensor("wv", [2, 97, 198], BF16, kind="ExternalInput").ap()
    wp_d = nc.dram_tensor("wp", [32, 6 * 192], BF16, kind="ExternalInput").ap()
    out_d = nc.dram_tensor("out", [WPC, N, C], F32, kind="ExternalOutput").ap()

    with tile.TileContext(nc) as tc:
        with (
            tc.tile_pool(name="static", bufs=1) as static_pool,
            tc.tile_pool(name="xa", bufs=3) as xa_pool,
            tc.tile_pool(name="ebm", bufs=3) as ebm_pool,
            tc.tile_pool(name="qk", bufs=2) as qk_pool,
            tc.tile_pool(name="vaug", bufs=4) as vaug_pool,
            tc.tile_pool(name="ep", bufs=2) as ep_pool,
            tc.tile_pool(name="zt", bufs=3) as zt_pool,
            tc.tile_pool(name="rr", bufs=3) as rr_pool,
            tc.tile_pool(name="spsum", bufs=1, space="PSUM") as s_psum,
            tc.tile_pool(name="opsum", bufs=3, space="PSUM") as o_psum,
            tc.tile_pool(name="ppsum", bufs=2, space="PSUM") as p_psum,
            tc.tile_pool(name="dram", bufs=3, space="DRAM") as dram_pool,
        ):
            # static weights
            wqk_t = []
            for kt in range(2):
                t = static_pool.tile([96, 384], BF16, tag=f"wqk{kt}")
                nc.sync.dma_start(t[:, :], wqk_d[kt])
                wqk_t.append(t)
            wv_t = []
            for kt in range(2):
                t = static_pool.tile([97, 198], BF16, tag=f"wv{kt}")
                nc.sync.dma_start(t[:, :], wv_d[kt])
                wv_t.append(t)
            wp_t = static_pool.tile([32, 6 * 192], BF16, tag="wp")
            nc.sync.dma_start(wp_t[:, :], wp_d[:, :])

            for w in range(WPC):
                # ---- input DMAs ----
                xa = xa_pool.tile([97, 392], BF16, tag="xa")
                nc.sync.dma_start(xa[:, :], xa_d[w])
                ebm = ebm_pool.tile([98, 6 * 392], BF16, tag="ebm")
                nc.sync.dma_start(ebm[:, :], ebm_d[w])

                # ---- qk projection: 3 feature tiles of 128 ----
                # feat f in [0,384): f<192 -> q head f//32, else k head
                qkps = []
                for ft in range(3):
                    ps = p_psum.tile([128, 512], F32, tag="ps2")
                    for kt in range(2):
                        nc.tensor.matmul(
                            ps[:, 0:196],
                            wqk_t[kt][:, ft * 128:(ft + 1) * 128],
                            xa[0:96, kt * 196:(kt + 1) * 196],
                            start=(kt == 0), stop=(kt == 1),
                        )
                    qkps.append(ps)

                # heads 0-2 in A tiles (rows 0-95), heads 3-5 in B tiles
                qT_A = qk_pool.tile([96, 196], BF16, tag="qTA")
                qT_B = qk_pool.tile([96, 196], BF16, tag="qTB")
                kT_A = qk_pool.tile([96, 196], BF16, tag="kTA")
                kT_B = qk_pool.tile([96, 196], BF16, tag="kTB")
                nc.vector.tensor_copy(qT_A[:, :], qkps[0][0:96, 0:196])
                nc.scalar.copy(qT_B[0:32, :], qkps[0][96:128, 0:196])
                nc.scalar.copy(qT_B[32:64, :], qkps[1][0:32, 0:196])
                nc.vector.tensor_copy(qT_B[64:96, :], qkps[1][32:64, 0:196])
                nc.vector.tensor_copy(kT_A[0:64, :], qkps[1][64:128, 0:196])
                nc.scalar.copy(kT_A[64:96, :], qkps[2][0:32, 0:196])
                nc.scalar.copy(kT_B[0:32, :], qkps[2][32:64, 0:196])
                nc.vector.tensor_copy(kT_B[32:64, :], qkps[2][64:96, 0:196])
                nc.scalar.copy(kT_B[64:96, :], qkps[2][96:128, 0:196])

                # ---- v projection (x stationary, ones-augmented) ----
                vaug = []
                for mt in range(2):
                    ps = p_psum.tile([128, 512], F32, tag="ps2")
                    for kt in range(2):
                        nc.tensor.matmul(
                            ps[0:98, 0:198],
                            xa[0:97, kt * 196 + mt * 98: kt * 196 + mt * 98 + 98],
                            wv_t[kt][:, :],
                            start=(kt == 0), stop=(kt == 1),
                        )
                    va = vaug_pool.tile([98, 198], BF16, tag="vaug")
                    nc.vector.tensor_copy(va[:, :], ps[0:98, 0:198])
                    vaug.append(va)

                # ---- per half (3 heads): QK^T -> S^T -> exp -> mul -> PV ----
                # half 0 = heads 0-2 (A tiles), half 1 = heads 3-5 (B tiles)
                s_sb = rr_pool.tile([1, 1176], F32, tag="ssb")
                P = ep_pool.tile([98, 6 * 392], BF16, tag="P")
                O_halves = []
                for hf in range(2):
                    kTh, qTh = (kT_A, qT_A) if hf == 0 else (kT_B, qT_B)
                    S = s_psum.tile([98, 3 * 512], F32, tag="S")
                    for hl in range(3):
                        for mt in range(2):
                            nc.tensor.matmul(
                                S[:, hl * 512 + mt * 196: hl * 512 + (mt + 1) * 196],
                                kTh[32 * hl:32 * hl + 32, mt * 98: mt * 98 + 98],
                                qTh[32 * hl:32 * hl + 32, :],
                                start=True, stop=True,
                            )
                    # exp across the 3 banks in one instr
                    E = ep_pool.tile([98, 3 * 392], BF16, tag="E")
                    S3 = S[:, :].rearrange("p (h x) -> p h x", h=3)[:, :, 0:392]
                    E3 = E[:, :].rearrange("p (h x) -> p h x", h=3)
                    nc.scalar.activation(E3, S3, mybir.ActivationFunctionType.Exp)
                    nc.vector.tensor_mul(
                        P[:, hf * 1176:(hf + 1) * 1176], E[:, :],
                        ebm[:, hf * 1176:(hf + 1) * 1176])

                    # PV into one 1-bank tile:
                    # local0: rows0-32 free0:196; local1: rows64-96 free0:196;
                    # local2: rows0-32 free196:392
                    O = o_psum.tile([98, 512], F32, tag="O")
                    O_halves.append(O)
                    for hl in range(3):
                        h = 3 * hf + hl
                        row = 64 if hl == 1 else 0
                        fo = 196 if hl == 2 else 0
                        for mt in range(2):
                            nc.tensor.matmul(
                                O[row:row + 33, fo:fo + 196],
                                vaug[mt][:, 33 * h: 33 * h + 33],
                                P[:, h * 392 + mt * 196: h * 392 + (mt + 1) * 196],
                                start=(mt == 0), stop=(mt == 1),
                            )
                    # s rows -> sbuf staging: [s_l0|s_l2] at row32, s_l1 at 96
                    nc.vector.tensor_copy(s_sb[0:1, hf * 588: hf * 588 + 392],
                                          O[32:33, 0:392])
                    nc.vector.tensor_copy(s_sb[0:1, hf * 588 + 392:(hf + 1) * 588],
                                          O[96:97, 0:196])

                # ---- reciprocal via DRAM transpose bounce ----
                scr = dram_pool.tile([1, 1176], F32, tag="scr")
                nc.gpsimd.dma_start(scr[0:1, :], s_sb[:, :])
                s_t = rr_pool.tile([98, 12], F32, tag="st")
                tr = AP(scr[:, :].tensor, scr[:, :].offset, [[12, 98], [1, 12]])
                nc.gpsimd.dma_start(s_t[:, :], tr)
                r_t = rr_pool.tile([98, 12], F32, tag="rt")
                nc.vector.reciprocal(r_t[:, :], s_t[:, :])
                scr2 = dram_pool.tile([1, 1176], F32, tag="scr2")
                tr2 = AP(scr2[:, :].tensor, scr2[:, :].offset, [[12, 98], [1, 12]])
                nc.gpsimd.dma_start(tr2, r_t[:, :])
                R_all = rr_pool.tile([32, 1176], F32, tag="Rall")
                bc = AP(scr2[:, :].tensor, scr2[:, :].offset, [[0, 32], [1, 1176]])
                nc.gpsimd.dma_start(R_all[:, :], bc)

                # ---- Z = O * r ---- (bf16 out; layout per half as in O)
                ztA = zt_pool.tile([32, 392], BF16, tag="ztA")  # h0, h2
                ztAm = zt_pool.tile([32, 196], BF16, tag="ztAm")  # h1
                ztB = zt_pool.tile([32, 392], BF16, tag="ztB")  # h3, h5
                ztBm = zt_pool.tile([32, 196], BF16, tag="ztBm")  # h4
                nc.vector.tensor_mul(ztA[:, :], O_halves[0][0:32, 0:392],
                                     R_all[:, 0:392])
                nc.vector.tensor_mul(ztAm[:, :], O_halves[0][64:96, 0:196],
                                     R_all[:, 392:588])
                nc.vector.tensor_mul(ztB[:, :], O_halves[1][0:32, 0:392],
                                     R_all[:, 588:980])
                nc.vector.tensor_mul(ztBm[:, :], O_halves[1][64:96, 0:196],
                                     R_all[:, 980:1176])

                # ---- projection: y[n_tile, c'] = sum_h Z_h^T.T @ wp_h ----
                zt_of = {0: (ztA, 0), 2: (ztA, 196), 1: (ztAm, 0),
                         3: (ztB, 0), 5: (ztB, 196), 4: (ztBm, 0)}
                for nt in range(2):
                    yp = o_psum.tile([98, 512], F32, tag="O")
                    for h in range(H):
                        zt_t, fo = zt_of[h]
                        nc.tensor.matmul(
                            yp[0:98, 0:192],
                            zt_t[:, fo + nt * 98: fo + nt * 98 + 98],
                            wp_t[:, h * 192:(h + 1) * 192],
                            start=(h == 0), stop=(h == 5),
                        )
                    y_sb = zt_pool.tile([98, 192], F32, tag="ysb")
                    if nt == 0:
                        nc.vector.tensor_copy(y_sb[:, :], yp[0:98, 0:192])
                    else:
                        nc.scalar.copy(y_sb[:, :], yp[0:98, 0:192])
                    nc.sync.dma_start(out_d[w, nt * 98:(nt + 1) * 98, :],
                                      y_sb[:, :])
    nc.compile()
    return nc


def _host_precompute(x, w_qkv, w_proj, bias_table, mask, rel_index):
    scale = HD ** (-0.5)
    wq = np.array(w_qkv, np.float32).copy()
    wq[0:C] *= scale  # fold softmax scale into q weights

    # xa[w, p, kt*196 + j] = x[w, j, kt*96 + p]; row 96: kt0->0, kt1->1
    xT = np.ascontiguousarray(np.transpose(np.asarray(x, np.float32), (0, 2, 1)))
    xa = np.zeros((B, 97, 392), np.float32)
    xa[:, 0:96, 0:196] = xT[:, 0:96]
    xa[:, 0:96, 196:392] = xT[:, 96:192]
    xa[:, 96, 196:392] = 1.0

    # wqk[kt, p, f] = wq[f, kt*96+p]  (f < 384: q then k features)
    wqkT = wq[0:384].T  # [192, 384]
    wqk = np.stack([wqkT[0:96], wqkT[96:192]])

    # wv[kt, p, 33h+d] = wq[384+32h+d, kt*96+p]; ones row kt1 p=96
    wv = np.zeros((2, 97, 198), np.float32)
    wvT = wq[384:576].T  # [192, 192] [c, (h,d)]
    for h in range(H):
        wv[0, 0:96, 33 * h: 33 * h + 32] = wvT[0:96, 32 * h: 32 * h + 32]
        wv[1, 0:96, 33 * h: 33 * h + 32] = wvT[96:192, 32 * h: 32 * h + 32]
        wv[1, 96, 33 * h + 32] = 1.0

    # wp[p, h*192 + c'] = w_proj[c', 32h + p]
    wp = np.zeros((32, 6 * 192), np.float32)
    wpT = np.asarray(w_proj, np.float32).T  # [c, c']
    for h in range(H):
        wp[:, h * 192:(h + 1) * 192] = wpT[32 * h: 32 * h + 32]

    # EBM[w, p, h*392 + mt*196 + n] = exp(bias[n, m, h] + mask[w, n, m]),
    # m = mt*98 + p
    bias = np.asarray(bias_table, np.float32)[np.asarray(rel_index).reshape(-1)]
    bias = bias.reshape(N, N, H)  # [n, m, h]
    biasT = np.transpose(bias, (2, 1, 0))  # [h, m, n]
    maskT = np.transpose(np.asarray(mask, np.float32), (0, 2, 1))  # [g, m, n]
    ebm = np.exp(biasT[None] + maskT[:, None])  # [g, h, m, n]
    ebm = ebm.reshape(NG, H, 2, MT, N).transpose(0, 3, 1, 2, 4)
    ebm = np.ascontiguousarray(ebm.reshape(NG, MT, H * 392))

    return (xa.astype(NPBF16), wqk.astype(NPBF16), wv.astype(NPBF16),
            wp.astype(NPBF16), ebm.astype(NPBF16))


def kernel(x, w_qkv, w_proj, b_proj, bias_table, mask, rel_index):
    xa, wqk, wv, wp, ebm = _host_precompute(
        x, w_qkv, w_proj, bias_table, mask, rel_index)

    if "nc" not in _CACHE:
        _CACHE["nc"] = _build_nc()
    nc = _CACHE["nc"]

    in_maps = []
    for c in range(NCORES):
        in_maps.append({
            "xa": np.ascontiguousarray(xa[c * WPC:(c + 1) * WPC]),
            "ebm": ebm,  # window w on core uses mask (64c+w) % 64 = w
            "wqk": wqk, "wv": wv, "wp": wp,
        })

    res = bass_utils.run_bass_kernel_spmd(nc, in_maps, core_ids=list(range(NCORES)))
    out = np.concatenate([res.results[c]["out"] for c in range(NCORES)], axis=0)
    out = out.astype(np.float32) + np.asarray(b_proj, np.float32)[None, None, :]
    return out


# revision 14
# speedup vs baseline: 1.6305x; 1.3338x over previous
"""Swin-style windowed attention on 8 TRN2 NeuronCores.

Data-parallel over windows: core i handles windows [64i, 64i+64).
Per-window device pipeline (S^T layout, m on partitions):
  qk-proj -> PSUM -> SBUF (qT/kT head-aligned)
  v-proj (x as stationary, ones-augmented weight cols) -> v_aug [98, 33*6]
  S^T = k^T.T @ qT per (head, m-tile) into 6 PSUM banks
  E = exp(S^T)               (one wide ACT instr across banks)
  P = E * exp(biasT+maskT)   (host-precomputed table, one wide DVE mul)
  O^T_aug = v_aug.T @ P      (per head+m-tile, accumulated; row 32 = softmax sum)
  r = 1/s ; broadcast over partitions via DRAM bounce ; Z = O * r
  y = Z^T.T @ w_proj.T per n-tile -> DMA out (f32)
Host: folds softmax scale into w_qkv, gathers rel-pos bias, builds EBM table,
adds b_proj at the end.
"""

import numpy as np
import ml_dtypes

import concourse.bass as bass
import concourse.mybir as mybir
import concourse.tile as tile
from concourse import bacc
from concourse import bass_utils
from concourse.bass import AP

BF16 = mybir.dt.bfloat16
F32 = mybir.dt.float32
NPBF16 = ml_dtypes.bfloat16

B, N, C, H, HD, NG = 512, 196, 192, 6, 32, 64
NCORES = 8
WPC = B // NCORES  # 64 windows per core
MT = 98            # m-tile size, 2 tiles cover N=196

_CACHE = {}


def _build_nc():
    nc = bacc.Bacc("TRN2", target_bir_lowering=False, debug=False,
                   enable_asserts=False)

    xa_d = nc.dram_tensor("xa", [WPC, 97, 392], BF16, kind="ExternalInput").ap()
    ebm_d = nc.dram_tensor("ebm", [WPC, 98, 6 * 392], BF16, kind="ExternalInput").ap()
    wqk_d = nc.dram_tensor("wqk", [2, 96, 384], BF16, kind="ExternalInput").ap()
    wv_d = nc.dram_tensor("wv", [2, 97, 198], BF16, kind="ExternalInput").ap()
    wp_d = nc.dram_tensor("wp", [32, 6 * 192], BF16, kind="ExternalInput").ap()
    out_d = nc.dram_tensor("out", [WPC, N, C], F32, kind="ExternalOutput").ap()

    with tile.TileContext(nc) as tc:
        with (
            tc.tile_pool(name="static", bufs=1) as static_pool,
            tc.tile_pool(name="xa", bufs=3) as xa_pool,
            tc.tile_pool(name="ebm", bufs=3) as ebm_pool,
            tc.tile_pool(name="qk", bufs=2) as qk_pool,
            tc.tile_pool(name="vaug", bufs=4) as vaug_pool,
            tc.tile_pool(name="ep", bufs=2) as ep_pool,
            tc.tile_pool(name="zt", bufs=3) as zt_pool,
            tc.tile_pool(name="rr", bufs=3) as rr_pool,
            tc.tile_pool(name="spsum", bufs=1, space="PSUM") as s_psum,
            tc.tile_pool(name="opsum", bufs=3, space="PSUM") as o_psum,
            tc.tile_pool(name="ppsum", bufs=2, space="PSUM") as p_psum,
            tc.tile_pool(name="dram", bufs=3, space="DRAM") as dram_pool,
        ):
            # static weights
            wqk_t = []
            for kt in range(2):
                t = static_pool.tile([96, 384], BF16, tag=f"wqk{kt}")
                nc.sync.dma_start(t[:, :], wqk_d[kt])
                wqk_t.append(t)
            wv_t = []
            for kt in range(2):
                t = static_pool.tile([97, 198], BF16, tag=f"wv{kt}")
                nc.sync.dma_start(t[:, :], wv_d[kt])
                wv_t.append(t)
            wp_t = static_pool.tile([32, 6 * 192], BF16, tag="wp")
            nc.sync.dma_start(wp_t[:, :], wp_d[:, :])

            for w in range(WPC):
                # ---- input DMAs ----
                xa = xa_pool.tile([97, 392], BF16, tag="xa")
                nc.sync.dma_start(xa[:, :], xa_d[w])
                ebm = ebm_pool.tile([98, 6 * 392], BF16, tag="ebm")
                nc.sync.dma_start(ebm[:, :], ebm_d[w])

                # ---- qk projection: 3 feature tiles of 128 ----
                # feat f in [0,384): f<192 -> q head f//32, else k head
                qkps = []
                for ft in range(3):
                    ps = p_psum.tile([128, 512], F32, tag="ps2")
                    for kt in range(2):
                        nc.tensor.matmul(
                            ps[:, 0:196],
                            wqk_t[kt][:, ft * 128:(ft + 1) * 128],
                            xa[0:96, kt * 196:(kt + 1) * 196],
                            start=(kt == 0), stop=(kt == 1),
                        )
                    qkps.append(ps)

                # heads 0-2 in A tiles (rows 0-95), heads 3-5 in B tiles
                qT_A = qk_pool.tile([96, 196], BF16, tag="qTA")
                qT_B = qk_pool.tile([96, 196], BF16, tag="qTB")
                kT_A = qk_pool.tile([96, 196], BF16, tag="kTA")
                kT_B = qk_pool.tile([96, 196], BF16, tag="kTB")
                nc.vector.tensor_copy(qT_A[:, :], qkps[0][0:96, 0:196])
                nc.scalar.copy(qT_B[0:32, :], qkps[0][96:128, 0:196])
                nc.scalar.copy(qT_B[32:64, :], qkps[1][0:32, 0:196])
                nc.vector.tensor_copy(qT_B[64:96, :], qkps[1][32:64, 0:196])
                nc.vector.tensor_copy(kT_A[0:64, :], qkps[1][64:128, 0:196])
                nc.scalar.copy(kT_A[64:96, :], qkps[2][0:32, 0:196])
                nc.scalar.copy(kT_B[0:32, :], qkps[2][32:64, 0:196])
                nc.vector.tensor_copy(kT_B[32:64, :], qkps[2][64:96, 0:196])
                nc.scalar.copy(kT_B[64:96, :], qkps[2][96:128, 0:196])

                # ---- v projection (x stationary, ones-augmented) ----
                vaug = []
                for mt in range(2):
                    ps = p_psum.tile([128, 512], F32, tag="ps2")
                    for kt in range(2):
                        nc.tensor.matmul(
                            ps[0:98, 0:198],
                            xa[0:97, kt * 196 + mt * 98: kt * 196 + mt * 98 + 98],
                            wv_t[kt][:, :],
                            start=(kt == 0), stop=(kt == 1),
                        )
                    va = vaug_pool.tile([98, 198], BF16, tag="vaug")
                    nc.vector.tensor_copy(va[:, :], ps[0:98, 0:198])
                    vaug.append(va)

                # ---- per half (3 heads): QK^T -> S^T -> exp -> mul -> PV ----
                # half 0 = heads 0-2 (A tiles), half 1 = heads 3-5 (B tiles)
                s_sb = rr_pool.tile([1, 1176], F32, tag="ssb")
                P = ep_pool.tile([98, 6 * 392], BF16, tag="P")
                O_halves = []
                for hf in range(2):
                    kTh, qTh = (kT_A, qT_A) if hf == 0 else (kT_B, qT_B)
                    S = s_psum.tile([98, 3 * 512], F32, tag="S")
                    for hl in range(3):
                        for mt in range(2):
                            nc.tensor.matmul(
                                S[:, hl * 512 + mt * 196: hl * 512 + (mt + 1) * 196],
                                kTh[32 * hl:32 * hl + 32, mt * 98: mt * 98 + 98],
                                qTh[32 * hl:32 * hl + 32, :],
                                start=True, stop=True,
                            )
                    # exp across the 3 banks in one instr
                    E = ep_pool.tile([98, 3 * 392], BF16, tag="E")
                    S3 = S[:, :].rearrange("p (h x) -> p h x", h=3)[:, :, 0:392]
                    E3 = E[:, :].rearrange("p (h x) -> p h x", h=3)
                    nc.scalar.activation(E3, S3, mybir.ActivationFunctionType.Exp)
                    nc.vector.tensor_mul(
                        P[:, hf * 1176:(hf + 1) * 1176], E[:, :],
                        ebm[:, hf * 1176:(hf + 1) * 1176])

                    # PV into one 1-bank tile:
                    # local0: rows0-32 free0:196; local1: rows64-96 free0:196;
                    # local2: rows0-32 free196:392
                    O = o_psum.tile([98, 512], F32, tag="O")
                    O_halves.append(O)
                    for hl in range(3):
                        h = 3 * hf + hl
                        row = 64 if hl == 1 else 0
                        fo = 196 if hl == 2 else 0
                        for mt in range(2):
                            nc.tensor.matmul(
                                O[row:row + 33, fo:fo + 196],
                                vaug[mt][:, 33 * h: 33 * h + 33],
                                P[:, h * 392 + mt * 196: h * 392 + (mt + 1) * 196],
                                start=(mt == 0), stop=(mt == 1),
                            )
                    # s rows -> sbuf staging: [s_l0|s_l2] at row32, s_l1 at 96
                    nc.vector.tensor_copy(s_sb[0:1, hf * 588: hf * 588 + 392],
                                          O[32:33, 0:392])
                    nc.vector.tensor_copy(s_sb[0:1, hf * 588 + 392:(hf + 1) * 588],
                                          O[96:97, 0:196])

                # ---- reciprocal: SBUF->SBUF transpose, recip on 98 lanes,
                #      transpose back, partition-broadcast on gpsimd ----
                s_t = rr_pool.tile([98, 12], F32, tag="st")
                nc.gpsimd.dma_start(s_t[:, :], s_sb[0:1, :])
                r_t = rr_pool.tile([98, 12], F32, tag="rt")
                nc.vector.reciprocal(r_t[:, :], s_t[:, :])
                r_row = rr_pool.tile([1, 1176], F32, tag="rrow")
                nc.gpsimd.dma_start(r_row[0:1, :], r_t[:, :])
                R_all = rr_pool.tile([32, 1176], F32, tag="Rall")
                nc.gpsimd.partition_broadcast(R_all[:, :], r_row[0:1, :])

                # ---- Z = O * r ---- (bf16 out; layout per half as in O)
                ztA = zt_pool.tile([32, 392], BF16, tag="ztA")  # h0, h2
                ztAm = zt_pool.tile([32, 196], BF16, tag="ztAm")  # h1
                ztB = zt_pool.tile([32, 392], BF16, tag="ztB")  # h3, h5
                ztBm = zt_pool.tile([32, 196], BF16, tag="ztBm")  # h4
                nc.vector.tensor_mul(ztA[:, :], O_halves[0][0:32, 0:392],
                                     R_all[:, 0:392])
                nc.vector.tensor_mul(ztAm[:, :], O_halves[0][64:96, 0:196],
                                     R_all[:, 392:588])
                nc.vector.tensor_mul(ztB[:, :], O_halves[1][0:32, 0:392],
                                     R_all[:, 588:980])
                nc.vector.tensor_mul(ztBm[:, :], O_halves[1][64:96, 0:196],
                                     R_all[:, 980:1176])

                # ---- projection: y[n_tile, c'] = sum_h Z_h^T.T @ wp_h ----
                zt_of = {0: (ztA, 0), 2: (ztA, 196), 1: (ztAm, 0),
                         3: (ztB, 0), 5: (ztB, 196), 4: (ztBm, 0)}
                for nt in range(2):
                    yp = o_psum.tile([98, 512], F32, tag="O")
                    for h in range(H):
                        zt_t, fo = zt_of[h]
                        nc.tensor.matmul(
                            yp[0:98, 0:192],
                            zt_t[:, fo + nt * 98: fo + nt * 98 + 98],
                            wp_t[:, h * 192:(h + 1) * 192],
                            start=(h == 0), stop=(h == 5),
                        )
                    y_sb = zt_pool.tile([98, 192], F32, tag="ysb")
                    if nt == 0:
                        nc.vector.tensor_copy(y_sb[:, :], yp[0:98, 0:192])
                    else:
                        nc.scalar.copy(y_sb[:, :], yp[0:98, 0:192])
                    nc.sync.dma_start(out_d[w, nt * 98:(nt + 1) * 98, :],
                                      y_sb[:, :])
    nc.compile()
    return nc


def _host_precompute(x, w_qkv, w_proj, bias_table, mask, rel_index):
    scale = HD ** (-0.5)
    wq = np.array(w_qkv, np.float32).copy()
    wq[0:C] *= scale  # fold softmax scale into q weights

    # xa[w, p, kt*196 + j] = x[w, j, kt*96 + p]; row 96: kt0->0, kt1->1
    xT = np.ascontiguousarray(np.transpose(np.asarray(x, np.float32), (0, 2, 1)))
    xa = np.zeros((B, 97, 392), np.float32)
    xa[:, 0:96, 0:196] = xT[:, 0:96]
    xa[:, 0:96, 196:392] = xT[:, 96:192]
    xa[:, 96, 196:392] = 1.0

    # wqk[kt, p, f] = wq[f, kt*96+p]  (f < 384: q then k features)
    wqkT = wq[0:384].T  # [192, 384]
    wqk = np.stack([wqkT[0:96], wqkT[96:192]])

    # wv[kt, p, 33h+d] = wq[384+32h+d, kt*96+p]; ones row kt1 p=96
    wv = np.zeros((2, 97, 198), np.float32)
    wvT = wq[384:576].T  # [192, 192] [c, (h,d)]
    for h in range(H):
        wv[0, 0:96, 33 * h: 33 * h + 32] = wvT[0:96, 32 * h: 32 * h + 32]
        wv[1, 0:96, 33 * h: 33 * h + 32] = wvT[96:192, 32 * h: 32 * h + 32]
        wv[1, 96, 33 * h + 32] = 1.0

    # wp[p, h*192 + c'] = w_proj[c', 32h + p]
    wp = np.zeros((32, 6 * 192), np.float32)
    wpT = np.asarray(w_proj, np.float32).T  # [c, c']
    for h in range(H):
        wp[:, h * 192:(h + 1) * 192] = wpT[32 * h: 32 * h + 32]

    # EBM[w, p, h*392 + mt*196 + n] = exp(bias[n, m, h] + mask[w, n, m]),
    # m = mt*98 + p
    bias = np.asarray(bias_table, np.float32)[np.asarray(rel_index).reshape(-1)]
    bias = bias.reshape(N, N, H)  # [n, m, h]
    biasT = np.transpose(bias, (2, 1, 0))  # [h, m, n]
    maskT = np.transpose(np.asarray(mask, np.float32), (0, 2, 1))  # [g, m, n]
    ebm = np.exp(biasT[None] + maskT[:, None])  # [g, h, m, n]
    ebm = ebm.reshape(NG, H, 2, MT, N).transpose(0, 3, 1, 2, 4)
    ebm = np.ascontiguousarray(ebm.reshape(NG, MT, H * 392))

    return (xa.astype(NPBF16), wqk.astype(NPBF16), wv.astype(NPBF16),
            wp.astype(NPBF16), ebm.astype(NPBF16))


def kernel(x, w_qkv, w_proj, b_proj, bias_table, mask, rel_index):
    xa, wqk, wv, wp, ebm = _host_precompute(
        x, w_qkv, w_proj, bias_table, mask, rel_index)

    if "nc" not in _CACHE:
        _CACHE["nc"] = _build_nc()
    nc = _CACHE["nc"]

    in_maps = []
    for c in range(NCORES):
        in_maps.append({
            "xa": np.ascontiguousarray(xa[c * WPC:(c + 1) * WPC]),
            "ebm": ebm,  # window w on core uses mask (64c+w) % 64 = w
            "wqk": wqk, "wv": wv, "wp": wp,
        })

    res = bass_utils.run_bass_kernel_spmd(nc, in_maps, core_ids=list(range(NCORES)))
    out = np.concatenate([res.results[c]["out"] for c in range(NCORES)], axis=0)
    out = out.astype(np.float32) + np.asarray(b_proj, np.float32)[None, None, :]
    return out


# revision 16
# speedup vs baseline: 1.6985x; 1.0418x over previous
"""Swin-style windowed attention on 8 TRN2 NeuronCores.

Data-parallel over windows: core i handles windows [64i, 64i+64).
Per-window device pipeline (S^T layout, m on partitions):
  qk-proj -> PSUM -> SBUF (qT/kT head-aligned)
  v-proj (x as stationary, ones-augmented weight cols) -> v_aug [98, 33*6]
  S^T = k^T.T @ qT per (head, m-tile) into 6 PSUM banks
  E = exp(S^T)               (one wide ACT instr across banks)
  P = E * exp(biasT+maskT)   (host-precomputed table, one wide DVE mul)
  O^T_aug = v_aug.T @ P      (per head+m-tile, accumulated; row 32 = softmax sum)
  r = 1/s ; broadcast over partitions via DRAM bounce ; Z = O * r
  y = Z^T.T @ w_proj.T per n-tile -> DMA out (f32)
Host: folds softmax scale into w_qkv, gathers rel-pos bias, builds EBM table,
adds b_proj at the end.
"""

import numpy as np
import ml_dtypes

import concourse.bass as bass
import concourse.mybir as mybir
import concourse.tile as tile
from concourse import bacc
from concourse import bass_utils
from concourse.bass import AP

BF16 = mybir.dt.bfloat16
F32 = mybir.dt.float32
NPBF16 = ml_dtypes.bfloat16

B, N, C, H, HD, NG = 512, 196, 192, 6, 32, 64
NCORES = 8
WPC = B // NCORES  # 64 windows per core
MT = 98            # m-tile size, 2 tiles cover N=196

_CACHE = {}


def _build_nc():
    nc = bacc.Bacc("TRN2", target_bir_lowering=False, debug=False,
                   enable_asserts=False)

    xa_d = nc.dram_tensor("xa", [WPC, 97, 392], BF16, kind="ExternalInput").ap()
    ebm_d = nc.dram_tensor("ebm", [WPC, 98, 6 * 392], BF16, kind="ExternalInput").ap()
    wqk_d = nc.dram_tensor("wqk", [2, 96, 384], BF16, kind="ExternalInput").ap()
    wv_d = nc.dram_tensor("wv", [2, 97, 198], BF16, kind="ExternalInput").ap()
    wp_d = nc.dram_tensor("wp", [2, 96, 192], BF16, kind="ExternalInput").ap()
    out_d = nc.dram_tensor("out", [WPC, N, C], F32, kind="ExternalOutput").ap()

    with tile.TileContext(nc) as tc:
        with (
            tc.tile_pool(name="static", bufs=1) as static_pool,
            tc.tile_pool(name="xa", bufs=3) as xa_pool,
            tc.tile_pool(name="ebm", bufs=3) as ebm_pool,
            tc.tile_pool(name="qk", bufs=2) as qk_pool,
            tc.tile_pool(name="vaug", bufs=4) as vaug_pool,
            tc.tile_pool(name="ep", bufs=2) as ep_pool,
            tc.tile_pool(name="zt", bufs=3) as zt_pool,
            tc.tile_pool(name="rr", bufs=3) as rr_pool,
            tc.tile_pool(name="spsum", bufs=1, space="PSUM") as s_psum,
            tc.tile_pool(name="opsum", bufs=3, space="PSUM") as o_psum,
            tc.tile_pool(name="ppsum", bufs=2, space="PSUM") as p_psum,
            tc.tile_pool(name="dram", bufs=3, space="DRAM") as dram_pool,
        ):
            # static weights
            wqk_t = []
            for kt in range(2):
                t = static_pool.tile([96, 384], BF16, tag=f"wqk{kt}")
                nc.sync.dma_start(t[:, :], wqk_d[kt])
                wqk_t.append(t)
            wv_t = []
            for kt in range(2):
                t = static_pool.tile([97, 198], BF16, tag=f"wv{kt}")
                nc.sync.dma_start(t[:, :], wv_d[kt])
                wv_t.append(t)
            wp_t = []
            for kt in range(2):
                t = static_pool.tile([96, 192], BF16, tag=f"wp{kt}")
                nc.sync.dma_start(t[:, :], wp_d[kt])
                wp_t.append(t)

            for w in range(WPC):
                # ---- input DMAs ----
                xa = xa_pool.tile([97, 392], BF16, tag="xa")
                nc.sync.dma_start(xa[:, :], xa_d[w])
                ebm = ebm_pool.tile([98, 6 * 392], BF16, tag="ebm")
                nc.sync.dma_start(ebm[:, :], ebm_d[w])

                # ---- qk projection: 4 feature tiles of 96 ----
                # ft0=q h0-2, ft1=q h3-5, ft2=k h0-2, ft3=k h3-5
                qk_sb = []
                for ft in range(4):
                    ps = p_psum.tile([128, 512], F32, tag="ps2")
                    for kt in range(2):
                        nc.tensor.matmul(
                            ps[0:96, 0:196],
                            wqk_t[kt][:, ft * 96:(ft + 1) * 96],
                            xa[0:96, kt * 196:(kt + 1) * 196],
                            start=(kt == 0), stop=(kt == 1),
                        )
                    t = qk_pool.tile([96, 196], BF16, tag=f"qk{ft}")
                    if ft % 2 == 0:
                        nc.vector.tensor_copy(t[:, :], ps[0:96, 0:196])
                    else:
                        nc.scalar.copy(t[:, :], ps[0:96, 0:196])
                    qk_sb.append(t)
                qT_A, qT_B, kT_A, kT_B = qk_sb

                # ---- v projection (x stationary, ones-augmented) ----
                vaug = []
                for mt in range(2):
                    ps = p_psum.tile([128, 512], F32, tag="ps2")
                    for kt in range(2):
                        nc.tensor.matmul(
                            ps[0:98, 0:198],
                            xa[0:97, kt * 196 + mt * 98: kt * 196 + mt * 98 + 98],
                            wv_t[kt][:, :],
                            start=(kt == 0), stop=(kt == 1),
                        )
                    va = vaug_pool.tile([98, 198], BF16, tag="vaug")
                    nc.vector.tensor_copy(va[:, :], ps[0:98, 0:198])
                    vaug.append(va)

                # ---- per half (3 heads): QK^T -> S^T -> exp -> mul -> PV ----
                # half 0 = heads 0-2 (A tiles), half 1 = heads 3-5 (B tiles)
                s_sb = rr_pool.tile([1, 1176], F32, tag="ssb")
                P = ep_pool.tile([98, 6 * 392], BF16, tag="P")
                O_halves = []
                for hf in range(2):
                    kTh, qTh = (kT_A, qT_A) if hf == 0 else (kT_B, qT_B)
                    S = s_psum.tile([98, 3 * 512], F32, tag="S")
                    for hl in range(3):
                        for mt in range(2):
                            nc.tensor.matmul(
                                S[:, hl * 512 + mt * 196: hl * 512 + (mt + 1) * 196],
                                kTh[32 * hl:32 * hl + 32, mt * 98: mt * 98 + 98],
                                qTh[32 * hl:32 * hl + 32, :],
                                start=True, stop=True,
                                tile_position=(32 * hl, 0),
                            )
                    # exp across the 3 banks in one instr
                    E = ep_pool.tile([98, 3 * 392], BF16, tag="E")
                    S3 = S[:, :].rearrange("p (h x) -> p h x", h=3)[:, :, 0:392]
                    E3 = E[:, :].rearrange("p (h x) -> p h x", h=3)
                    nc.scalar.activation(E3, S3, mybir.ActivationFunctionType.Exp)
                    nc.vector.tensor_mul(
                        P[:, hf * 1176:(hf + 1) * 1176], E[:, :],
                        ebm[:, hf * 1176:(hf + 1) * 1176])

                    # PV into one 1-bank tile:
                    # local0: rows0-32 free0:196; local1: rows64-96 free0:196;
                    # local2: rows0-32 free196:392
                    O = o_psum.tile([98, 512], F32, tag="O")
                    O_halves.append(O)
                    for hl in range(3):
                        h = 3 * hf + hl
                        row = 64 if hl == 1 else 0
                        fo = 196 if hl == 2 else 0
                        for mt in range(2):
                            nc.tensor.matmul(
                                O[row:row + 33, fo:fo + 196],
                                vaug[mt][:, 33 * h: 33 * h + 33],
                                P[:, h * 392 + mt * 196: h * 392 + (mt + 1) * 196],
                                start=(mt == 0), stop=(mt == 1),
                            )
                    # s rows -> sbuf staging: [s_l0|s_l2] at row32, s_l1 at 96
                    nc.vector.tensor_copy(s_sb[0:1, hf * 588: hf * 588 + 392],
                                          O[32:33, 0:392])
                    nc.vector.tensor_copy(s_sb[0:1, hf * 588 + 392:(hf + 1) * 588],
                                          O[96:97, 0:196])

                # ---- reciprocal: SBUF->SBUF transpose, recip on 98 lanes,
                #      transpose back, partition-broadcast on gpsimd ----
                s_t = rr_pool.tile([98, 12], F32, tag="st")
                nc.gpsimd.dma_start(s_t[:, :], s_sb[0:1, :])
                r_t = rr_pool.tile([98, 12], F32, tag="rt")
                nc.vector.reciprocal(r_t[:, :], s_t[:, :])
                r_row = rr_pool.tile([1, 1176], F32, tag="rrow")
                nc.gpsimd.dma_start(r_row[0:1, :], r_t[:, :])
                R_all = rr_pool.tile([32, 1176], F32, tag="Rall")
                nc.gpsimd.partition_broadcast(R_all[:, :], r_row[0:1, :])

                # ---- Z = O * r -> Z^T chunks [96,196] (heads at rows 0/32/64)
                zt0 = zt_pool.tile([96, 196], BF16, tag="zt0")
                zt1 = zt_pool.tile([96, 196], BF16, tag="zt1")
                zt = [zt0, zt1]
                nc.vector.tensor_mul(zt[0][0:32, :], O_halves[0][0:32, 0:196],
                                     R_all[:, 0:196])
                nc.vector.tensor_mul(zt[0][64:96, :], O_halves[0][0:32, 196:392],
                                     R_all[:, 196:392])
                nc.vector.tensor_mul(zt[0][32:64, :], O_halves[0][64:96, 0:196],
                                     R_all[:, 392:588])
                nc.vector.tensor_mul(zt[1][0:32, :], O_halves[1][0:32, 0:196],
                                     R_all[:, 588:784])
                nc.vector.tensor_mul(zt[1][64:96, :], O_halves[1][0:32, 196:392],
                                     R_all[:, 784:980])
                nc.vector.tensor_mul(zt[1][32:64, :], O_halves[1][64:96, 0:196],
                                     R_all[:, 980:1176])

                # ---- projection: y[n_tile, c'] = sum_kt Z^T_kt.T @ wp_kt ----
                for nt in range(2):
                    yp = o_psum.tile([98, 512], F32, tag="O")
                    for kt in range(2):
                        nc.tensor.matmul(
                            yp[0:98, 0:192],
                            zt[kt][:, nt * 98: nt * 98 + 98],
                            wp_t[kt][:, :],
                            start=(kt == 0), stop=(kt == 1),
                        )
                    y_sb = zt_pool.tile([98, 192], F32, tag="ysb")
                    if nt == 0:
                        nc.vector.tensor_copy(y_sb[:, :], yp[0:98, 0:192])
                    else:
                        nc.scalar.copy(y_sb[:, :], yp[0:98, 0:192])
                    nc.sync.dma_start(out_d[w, nt * 98:(nt + 1) * 98, :],
                                      y_sb[:, :])
    nc.compile()
    return nc


def _host_precompute(x, w_qkv, w_proj, bias_table, mask, rel_index):
    scale = HD ** (-0.5)
    wq = np.array(w_qkv, np.float32).copy()
    wq[0:C] *= scale  # fold softmax scale into q weights

    # xa[w, p, kt*196 + j] = x[w, j, kt*96 + p]; row 96: kt0->0, kt1->1
    xT = np.ascontiguousarray(np.transpose(np.asarray(x, np.float32), (0, 2, 1)))
    xa = np.zeros((B, 97, 392), np.float32)
    xa[:, 0:96, 0:196] = xT[:, 0:96]
    xa[:, 0:96, 196:392] = xT[:, 96:192]
    xa[:, 96, 196:392] = 1.0

    # wqk[kt, p, f] = wq[f, kt*96+p]  (f < 384: q then k features)
    wqkT = wq[0:384].T  # [192, 384]
    wqk = np.stack([wqkT[0:96], wqkT[96:192]])

    # wv[kt, p, 33h+d] = wq[384+32h+d, kt*96+p]; ones row kt1 p=96
    wv = np.zeros((2, 97, 198), np.float32)
    wvT = wq[384:576].T  # [192, 192] [c, (h,d)]
    for h in range(H):
        wv[0, 0:96, 33 * h: 33 * h + 32] = wvT[0:96, 32 * h: 32 * h + 32]
        wv[1, 0:96, 33 * h: 33 * h + 32] = wvT[96:192, 32 * h: 32 * h + 32]
        wv[1, 96, 33 * h + 32] = 1.0

    # wp[kt, p, c'] = w_proj[c', head_perm...] matching zt row order
    wp = np.zeros((2, 96, 192), np.float32)
    wpT = np.asarray(w_proj, np.float32).T  # [c, c']
    head_rows = {0: (0, 0), 2: (0, 64), 1: (0, 32),
                 3: (1, 0), 5: (1, 64), 4: (1, 32)}
    for h, (kt, row) in head_rows.items():
        wp[kt, row:row + 32, :] = wpT[32 * h: 32 * h + 32]

    # EBM[w, p, h*392 + mt*196 + n] = exp(bias[n, m, h] + mask[w, n, m]),
    # m = mt*98 + p
    bias = np.asarray(bias_table, np.float32)[np.asarray(rel_index).reshape(-1)]
    bias = bias.reshape(N, N, H)  # [n, m, h]
    biasT = np.transpose(bias, (2, 1, 0))  # [h, m, n]
    maskT = np.transpose(np.asarray(mask, np.float32), (0, 2, 1))  # [g, m, n]
    ebm = np.exp(biasT[None] + maskT[:, None])  # [g, h, m, n]
    ebm = ebm.reshape(NG, H, 2, MT, N).transpose(0, 3, 1, 2, 4)
    ebm = np.ascontiguousarray(ebm.reshape(NG, MT, H * 392))

    return (xa.astype(NPBF16), wqk.astype(NPBF16), wv.astype(NPBF16),
            wp.astype(NPBF16), ebm.astype(NPBF16))


def kernel(x, w_qkv, w_proj, b_proj, bias_table, mask, rel_index):
    xa, wqk, wv, wp, ebm = _host_precompute(
        x, w_qkv, w_proj, bias_table, mask, rel_index)

    if "nc" not in _CACHE:
        _CACHE["nc"] = _build_nc()
    nc = _CACHE["nc"]

    in_maps = []
    for c in range(NCORES):
        in_maps.append({
            "xa": np.ascontiguousarray(xa[c * WPC:(c + 1) * WPC]),
            "ebm": ebm,  # window w on core uses mask (64c+w) % 64 = w
            "wqk": wqk, "wv": wv, "wp": wp,
        })

    res = bass_utils.run_bass_kernel_spmd(nc, in_maps, core_ids=list(range(NCORES)))
    out = np.concatenate([res.results[c]["out"] for c in range(NCORES)], axis=0)
    out = out.astype(np.float32) + np.asarray(b_proj, np.float32)[None, None, :]
    return out


# revision 18
# speedup vs baseline: 1.7175x; 1.0112x over previous
"""Swin-style windowed attention on 8 TRN2 NeuronCores.

Data-parallel over windows: core i handles windows [64i, 64i+64).
Per-window device pipeline (S^T layout, m on partitions):
  qk-proj -> PSUM -> SBUF (qT/kT head-aligned)
  v-proj (x as stationary, ones-augmented weight cols) -> v_aug [98, 33*6]
  S^T = k^T.T @ qT per (head, m-tile) into 6 PSUM banks
  E = exp(S^T)               (one wide ACT instr across banks)
  P = E * exp(biasT+maskT)   (host-precomputed table, one wide DVE mul)
  O^T_aug = v_aug.T @ P      (per head+m-tile, accumulated; row 32 = softmax sum)
  r = 1/s ; broadcast over partitions via DRAM bounce ; Z = O * r
  y = Z^T.T @ w_proj.T per n-tile -> DMA out (f32)
Host: folds softmax scale into w_qkv, gathers rel-pos bias, builds EBM table,
adds b_proj at the end.
"""

import numpy as np
import ml_dtypes

import concourse.bass as bass
import concourse.mybir as mybir
import concourse.tile as tile
from concourse import bacc
from concourse import bass_utils
from concourse.bass import AP

BF16 = mybir.dt.bfloat16
F32 = mybir.dt.float32
NPBF16 = ml_dtypes.bfloat16

B, N, C, H, HD, NG = 512, 196, 192, 6, 32, 64
NCORES = 8
WPC = B // NCORES  # 64 windows per core
MT = 98            # m-tile size, 2 tiles cover N=196

_CACHE = {}


def _build_nc():
    nc = bacc.Bacc("TRN2", target_bir_lowering=False, debug=False,
                   enable_asserts=False)

    xa_d = nc.dram_tensor("xa", [WPC, 97, 392], BF16, kind="ExternalInput").ap()
    ebm_d = nc.dram_tensor("ebm", [WPC, 98, 6 * 392], BF16, kind="ExternalInput").ap()
    wqk_d = nc.dram_tensor("wqk", [2, 96, 384], BF16, kind="ExternalInput").ap()
    wv_d = nc.dram_tensor("wv", [2, 97, 198], BF16, kind="ExternalInput").ap()
    wp_d = nc.dram_tensor("wp", [2, 96, 192], BF16, kind="ExternalInput").ap()
    out_d = nc.dram_tensor("out", [WPC, N, C], F32, kind="ExternalOutput").ap()

    with tile.TileContext(nc) as tc:
        with (
            tc.tile_pool(name="static", bufs=1) as static_pool,
            tc.tile_pool(name="xa", bufs=3) as xa_pool,
            tc.tile_pool(name="ebm", bufs=3) as ebm_pool,
            tc.tile_pool(name="qk", bufs=2) as qk_pool,
            tc.tile_pool(name="vaug", bufs=4) as vaug_pool,
            tc.tile_pool(name="ep", bufs=2) as ep_pool,
            tc.tile_pool(name="zt", bufs=3) as zt_pool,
            tc.tile_pool(name="rr", bufs=3) as rr_pool,
            tc.tile_pool(name="spsum", bufs=1, space="PSUM") as s_psum,
            tc.tile_pool(name="opsum", bufs=3, space="PSUM") as o_psum,
            tc.tile_pool(name="ppsum", bufs=2, space="PSUM") as p_psum,
            tc.tile_pool(name="dram", bufs=3, space="DRAM") as dram_pool,
        ):
            # static weights
            wqk_t = []
            for kt in range(2):
                t = static_pool.tile([96, 384], BF16, tag=f"wqk{kt}")
                nc.sync.dma_start(t[:, :], wqk_d[kt])
                wqk_t.append(t)
            wv_t = []
            for kt in range(2):
                t = static_pool.tile([97, 198], BF16, tag=f"wv{kt}")
                nc.sync.dma_start(t[:, :], wv_d[kt])
                wv_t.append(t)
            wp_t = []
            for kt in range(2):
                t = static_pool.tile([96, 192], BF16, tag=f"wp{kt}")
                nc.sync.dma_start(t[:, :], wp_d[kt])
                wp_t.append(t)

            for w in range(WPC):
                # ---- input DMAs ----
                xa = xa_pool.tile([97, 392], BF16, tag="xa")
                nc.sync.dma_start(xa[:, :], xa_d[w])
                ebm = ebm_pool.tile([98, 6 * 392], BF16, tag="ebm")
                nc.sync.dma_start(ebm[:, :], ebm_d[w])

                # ---- qk projection: 4 feature tiles of 96 ----
                # ft0=q h0-2, ft1=q h3-5, ft2=k h0-2, ft3=k h3-5
                qk_sb = []
                for ft in range(4):
                    ps = p_psum.tile([128, 512], F32, tag="ps2")
                    for kt in range(2):
                        nc.tensor.matmul(
                            ps[0:96, 0:196],
                            wqk_t[kt][:, ft * 96:(ft + 1) * 96],
                            xa[0:96, kt * 196:(kt + 1) * 196],
                            start=(kt == 0), stop=(kt == 1),
                        )
                    t = qk_pool.tile([96, 196], BF16, tag=f"qk{ft}")
                    if ft % 2 == 0:
                        nc.vector.tensor_copy(t[:, :], ps[0:96, 0:196])
                    else:
                        nc.scalar.copy(t[:, :], ps[0:96, 0:196])
                    qk_sb.append(t)
                qT_A, qT_B, kT_A, kT_B = qk_sb

                # ---- v projection (x stationary, ones-augmented) ----
                vaug = []
                for mt in range(2):
                    ps = p_psum.tile([128, 512], F32, tag="ps2")
                    for kt in range(2):
                        nc.tensor.matmul(
                            ps[0:98, 0:198],
                            xa[0:97, kt * 196 + mt * 98: kt * 196 + mt * 98 + 98],
                            wv_t[kt][:, :],
                            start=(kt == 0), stop=(kt == 1),
                        )
                    va = vaug_pool.tile([98, 198], BF16, tag="vaug")
                    nc.vector.tensor_copy(va[:, :], ps[0:98, 0:198])
                    vaug.append(va)

                # ---- per half (3 heads): QK^T -> S^T -> exp -> mul -> PV ----
                # half 0 = heads 0-2 (A tiles), half 1 = heads 3-5 (B tiles)
                s_sb = rr_pool.tile([1, 1176], F32, tag="ssb")
                U_halves = []
                P = ep_pool.tile([98, 6 * 392], BF16, tag="P")
                O_halves = []
                for hf in range(2):
                    kTh, qTh = (kT_A, qT_A) if hf == 0 else (kT_B, qT_B)
                    S = s_psum.tile([98, 3 * 512], F32, tag="S")
                    for hl in range(3):
                        for mt in range(2):
                            nc.tensor.matmul(
                                S[:, hl * 512 + mt * 196: hl * 512 + (mt + 1) * 196],
                                kTh[32 * hl:32 * hl + 32, mt * 98: mt * 98 + 98],
                                qTh[32 * hl:32 * hl + 32, :],
                                start=True, stop=True,
                                tile_position=(32 * hl, 0),
                            )
                    # exp across the 3 banks in one instr
                    E = ep_pool.tile([98, 3 * 392], BF16, tag="E")
                    S3 = S[:, :].rearrange("p (h x) -> p h x", h=3)[:, :, 0:392]
                    E3 = E[:, :].rearrange("p (h x) -> p h x", h=3)
                    nc.scalar.activation(E3, S3, mybir.ActivationFunctionType.Exp)
                    nc.vector.tensor_mul(
                        P[:, hf * 1176:(hf + 1) * 1176], E[:, :],
                        ebm[:, hf * 1176:(hf + 1) * 1176])

                    # PV into one 1-bank tile:
                    # local0: rows0-32 free0:196; local1: rows64-96 free0:196;
                    # local2: rows0-32 free196:392
                    O = o_psum.tile([98, 512], F32, tag="O")
                    O_halves.append(O)
                    for hl in range(3):
                        h = 3 * hf + hl
                        row = 64 if hl == 1 else 0
                        fo = 196 if hl == 2 else 0
                        for mt in range(2):
                            nc.tensor.matmul(
                                O[row:row + 33, fo:fo + 196],
                                vaug[mt][:, 33 * h: 33 * h + 33],
                                P[:, h * 392 + mt * 196: h * 392 + (mt + 1) * 196],
                                start=(mt == 0), stop=(mt == 1),
                            )
                    # copy O (incl s rows) to SBUF to free PSUM quickly
                    U_e = zt_pool.tile([33, 392], BF16, tag="Ue")
                    U_o = zt_pool.tile([33, 196], BF16, tag="Uo")
                    if hf == 0:
                        nc.scalar.copy(U_e[:, :], O[0:33, 0:392])
                        nc.vector.tensor_copy(U_o[:, :], O[64:97, 0:196])
                    else:
                        nc.vector.tensor_copy(U_e[:, :], O[0:33, 0:392])
                        nc.scalar.copy(U_o[:, :], O[64:97, 0:196])
                    U_halves.append((U_e, U_o))
                    # s rows -> staging: [s_l0|s_l2] at U_e row32, s_l1 at U_o
                    nc.vector.tensor_copy(s_sb[0:1, hf * 588: hf * 588 + 392],
                                          U_e[32:33, :])
                    nc.vector.tensor_copy(s_sb[0:1, hf * 588 + 392:(hf + 1) * 588],
                                          U_o[32:33, :])

                # ---- reciprocal: SBUF->SBUF transpose, recip on 98 lanes,
                #      transpose back, partition-broadcast on gpsimd ----
                s_t = rr_pool.tile([98, 12], F32, tag="st")
                nc.gpsimd.dma_start(s_t[:, :], s_sb[0:1, :])
                r_t = rr_pool.tile([98, 12], BF16, tag="rt")
                with nc.allow_low_precision(reason="softmax recip; rel_err gate 2e-2"):
                    nc.vector.reciprocal(r_t[:, :], s_t[:, :])
                r_row = rr_pool.tile([1, 1176], BF16, tag="rrow")
                nc.gpsimd.dma_start(r_row[0:1, :], r_t[:, :])
                R_all = rr_pool.tile([32, 1176], BF16, tag="Rall")
                nc.gpsimd.partition_broadcast(R_all[:, :], r_row[0:1, :])

                # ---- Z = O * r -> Z^T chunks [96,196] (heads at rows 0/32/64)
                zt0 = zt_pool.tile([96, 196], BF16, tag="zt0")
                zt1 = zt_pool.tile([96, 196], BF16, tag="zt1")
                zt = [zt0, zt1]
                nc.vector.tensor_mul(zt[0][0:32, :], U_halves[0][0][0:32, 0:196],
                                     R_all[:, 0:196])
                nc.vector.tensor_mul(zt[0][64:96, :], U_halves[0][0][0:32, 196:392],
                                     R_all[:, 196:392])
                nc.vector.tensor_mul(zt[0][32:64, :], U_halves[0][1][0:32, :],
                                     R_all[:, 392:588])
                nc.vector.tensor_mul(zt[1][0:32, :], U_halves[1][0][0:32, 0:196],
                                     R_all[:, 588:784])
                nc.vector.tensor_mul(zt[1][64:96, :], U_halves[1][0][0:32, 196:392],
                                     R_all[:, 784:980])
                nc.vector.tensor_mul(zt[1][32:64, :], U_halves[1][1][0:32, :],
                                     R_all[:, 980:1176])

                # ---- projection: y[n_tile, c'] = sum_kt Z^T_kt.T @ wp_kt ----
                for nt in range(2):
                    yp = o_psum.tile([98, 512], F32, tag="O")
                    for kt in range(2):
                        nc.tensor.matmul(
                            yp[0:98, 0:192],
                            zt[kt][:, nt * 98: nt * 98 + 98],
                            wp_t[kt][:, :],
                            start=(kt == 0), stop=(kt == 1),
                        )
                    y_sb = zt_pool.tile([98, 192], F32, tag="ysb")
                    if nt == 0:
                        nc.vector.tensor_copy(y_sb[:, :], yp[0:98, 0:192])
                    else:
                        nc.scalar.copy(y_sb[:, :], yp[0:98, 0:192])
                    nc.sync.dma_start(out_d[w, nt * 98:(nt + 1) * 98, :],
                                      y_sb[:, :])
    nc.compile()
    return nc


def _host_precompute(x, w_qkv, w_proj, bias_table, mask, rel_index):
    scale = HD ** (-0.5)
    wq = np.array(w_qkv, np.float32).copy()
    wq[0:C] *= scale  # fold softmax scale into q weights

    # xa[w, p, kt*196 + j] = x[w, j, kt*96 + p]; row 96: kt0->0, kt1->1
    xT = np.ascontiguousarray(np.transpose(np.asarray(x, np.float32), (0, 2, 1)))
    xa = np.zeros((B, 97, 392), np.float32)
    xa[:, 0:96, 0:196] = xT[:, 0:96]
    xa[:, 0:96, 196:392] = xT[:, 96:192]
    xa[:, 96, 196:392] = 1.0

    # wqk[kt, p, f] = wq[f, kt*96+p]  (f < 384: q then k features)
    wqkT = wq[0:384].T  # [192, 384]
    wqk = np.stack([wqkT[0:96], wqkT[96:192]])

    # wv[kt, p, 33h+d] = wq[384+32h+d, kt*96+p]; ones row kt1 p=96
    wv = np.zeros((2, 97, 198), np.float32)
    wvT = wq[384:576].T  # [192, 192] [c, (h,d)]
    for h in range(H):
        wv[0, 0:96, 33 * h: 33 * h + 32] = wvT[0:96, 32 * h: 32 * h + 32]
        wv[1, 0:96, 33 * h: 33 * h + 32] = wvT[96:192, 32 * h: 32 * h + 32]
        wv[1, 96, 33 * h + 32] = 1.0

    # wp[kt, p, c'] = w_proj[c', head_perm...] matching zt row order
    wp = np.zeros((2, 96, 192), np.float32)
    wpT = np.asarray(w_proj, np.float32).T  # [c, c']
    head_rows = {0: (0, 0), 2: (0, 64), 1: (0, 32),
                 3: (1, 0), 5: (1, 64), 4: (1, 32)}
    for h, (kt, row) in head_rows.items():
        wp[kt, row:row + 32, :] = wpT[32 * h: 32 * h + 32]

    # EBM[w, p, h*392 + mt*196 + n] = exp(bias[n, m, h] + mask[w, n, m]),
    # m = mt*98 + p
    bias = np.asarray(bias_table, np.float32)[np.asarray(rel_index).reshape(-1)]
    bias = bias.reshape(N, N, H)  # [n, m, h]
    biasT = np.transpose(bias, (2, 1, 0))  # [h, m, n]
    maskT = np.transpose(np.asarray(mask, np.float32), (0, 2, 1))  # [g, m, n]
    ebm = np.exp(biasT[None] + maskT[:, None])  # [g, h, m, n]
    ebm = ebm.reshape(NG, H, 2, MT, N).transpose(0, 3, 1, 2, 4)
    ebm = np.ascontiguousarray(ebm.reshape(NG, MT, H * 392))

    return (xa.astype(NPBF16), wqk.astype(NPBF16), wv.astype(NPBF16),
            wp.astype(NPBF16), ebm.astype(NPBF16))


def kernel(x, w_qkv, w_proj, b_proj, bias_table, mask, rel_index):
    xa, wqk, wv, wp, ebm = _host_precompute(
        x, w_qkv, w_proj, bias_table, mask, rel_index)

    if "nc" not in _CACHE:
        _CACHE["nc"] = _build_nc()
    nc = _CACHE["nc"]

    in_maps = []
    for c in range(NCORES):
        in_maps.append({
            "xa": np.ascontiguousarray(xa[c * WPC:(c + 1) * WPC]),
            "ebm": ebm,  # window w on core uses mask (64c+w) % 64 = w
            "wqk": wqk, "wv": wv, "wp": wp,
        })

    res = bass_utils.run_bass_kernel_spmd(nc, in_maps, core_ids=list(range(NCORES)))
    out = np.concatenate([res.results[c]["out"] for c in range(NCORES)], axis=0)
    out = out.astype(np.float32) + np.asarray(b_proj, np.float32)[None, None, :]
    return out


# revision 19
# speedup vs baseline: 2.5340x; 1.4754x over previous
"""Swin-style windowed attention on 8 TRN2 NeuronCores.

Data-parallel over windows: core i handles windows [64i, 64i+64).
Per-window device pipeline (S^T layout, m on partitions):
  qk-proj -> PSUM -> SBUF (qT/kT head-aligned)
  v-proj (x as stationary, ones-augmented weight cols) -> v_aug [98, 33*6]
  S^T = k^T.T @ qT per (head, m-tile) into 6 PSUM banks
  E = exp(S^T)               (one wide ACT instr across banks)
  P = E * exp(biasT+maskT)   (host-precomputed table, one wide DVE mul)
  O^T_aug = v_aug.T @ P      (per head+m-tile, accumulated; row 32 = softmax sum)
  r = 1/s ; broadcast over partitions via DRAM bounce ; Z = O * r
  y = Z^T.T @ w_proj.T per n-tile -> DMA out (f32)
Host: folds softmax scale into w_qkv, gathers rel-pos bias, builds EBM table,
adds b_proj at the end.
"""

import numpy as np
import ml_dtypes

import concourse.bass as bass
import concourse.mybir as mybir
import concourse.tile as tile
from concourse import bacc
from concourse import bass_utils
from concourse.bass import AP

BF16 = mybir.dt.bfloat16
F32 = mybir.dt.float32
NPBF16 = ml_dtypes.bfloat16

B, N, C, H, HD, NG = 512, 196, 192, 6, 32, 64
NCORES = 8
WPC = B // NCORES  # 64 windows per core
MT = 98            # m-tile size, 2 tiles cover N=196

_CACHE = {}


def _build_nc():
    nc = bacc.Bacc("TRN2", target_bir_lowering=False, debug=False,
                   enable_asserts=False)

    xa_d = nc.dram_tensor("xa", [WPC, 97, 392], BF16, kind="ExternalInput").ap()
    ebm_d = nc.dram_tensor("ebm", [WPC, 98, 6 * 392], BF16, kind="ExternalInput").ap()
    wqk_d = nc.dram_tensor("wqk", [2, 96, 384], BF16, kind="ExternalInput").ap()
    wv_d = nc.dram_tensor("wv", [2, 97, 198], BF16, kind="ExternalInput").ap()
    wp_d = nc.dram_tensor("wp", [2, 96, 192], BF16, kind="ExternalInput").ap()
    out_d = nc.dram_tensor("out", [WPC, N, C], F32, kind="ExternalOutput").ap()

    with tile.TileContext(nc) as tc:
        with (
            tc.tile_pool(name="static", bufs=1) as static_pool,
            tc.tile_pool(name="xa", bufs=3) as xa_pool,
            tc.tile_pool(name="ebm", bufs=3) as ebm_pool,
            tc.tile_pool(name="qk", bufs=2) as qk_pool,
            tc.tile_pool(name="vaug", bufs=4) as vaug_pool,
            tc.tile_pool(name="ep", bufs=2) as ep_pool,
            tc.tile_pool(name="zt", bufs=6) as zt_pool,
            tc.tile_pool(name="rr", bufs=3) as rr_pool,
            tc.tile_pool(name="spsum", bufs=1, space="PSUM") as s_psum,
            tc.tile_pool(name="opsum", bufs=3, space="PSUM") as o_psum,
            tc.tile_pool(name="ppsum", bufs=2, space="PSUM") as p_psum,
            tc.tile_pool(name="dram", bufs=3, space="DRAM") as dram_pool,
        ):
            # static weights
            wqk_t = []
            for kt in range(2):
                t = static_pool.tile([96, 384], BF16, tag=f"wqk{kt}")
                nc.sync.dma_start(t[:, :], wqk_d[kt])
                wqk_t.append(t)
            wv_t = []
            for kt in range(2):
                t = static_pool.tile([97, 198], BF16, tag=f"wv{kt}")
                nc.sync.dma_start(t[:, :], wv_d[kt])
                wv_t.append(t)
            wp_t = []
            for kt in range(2):
                t = static_pool.tile([96, 192], BF16, tag=f"wp{kt}")
                nc.sync.dma_start(t[:, :], wp_d[kt])
                wp_t.append(t)

            carried = {}

            def back(st):
                U_halves, R_all, w = st["U"], st["R"], st["w"]
                # ---- Z = O * r -> Z^T chunks [96,196] (heads at rows 0/32/64)
                zt0 = zt_pool.tile([96, 196], BF16, tag="zt0")
                zt1 = zt_pool.tile([96, 196], BF16, tag="zt1")
                zt = [zt0, zt1]
                nc.vector.tensor_mul(zt[0][0:32, :], U_halves[0][0][0:32, 0:196],
                                     R_all[:, 0:196])
                nc.vector.tensor_mul(zt[0][64:96, :], U_halves[0][0][0:32, 196:392],
                                     R_all[:, 196:392])
                nc.vector.tensor_mul(zt[0][32:64, :], U_halves[0][1][0:32, :],
                                     R_all[:, 392:588])
                nc.vector.tensor_mul(zt[1][0:32, :], U_halves[1][0][0:32, 0:196],
                                     R_all[:, 588:784])
                nc.vector.tensor_mul(zt[1][64:96, :], U_halves[1][0][0:32, 196:392],
                                     R_all[:, 784:980])
                nc.vector.tensor_mul(zt[1][32:64, :], U_halves[1][1][0:32, :],
                                     R_all[:, 980:1176])

                # ---- projection: y[n_tile, c'] = sum_kt Z^T_kt.T @ wp_kt ----
                for nt in range(2):
                    yp = o_psum.tile([98, 512], F32, tag="O")
                    for kt in range(2):
                        nc.tensor.matmul(
                            yp[0:98, 0:192],
                            zt[kt][:, nt * 98: nt * 98 + 98],
                            wp_t[kt][:, :],
                            start=(kt == 0), stop=(kt == 1),
                        )
                    y_sb = zt_pool.tile([98, 192], F32, tag="ysb")
                    if nt == 0:
                        nc.vector.tensor_copy(y_sb[:, :], yp[0:98, 0:192])
                    else:
                        nc.scalar.copy(y_sb[:, :], yp[0:98, 0:192])
                    nc.sync.dma_start(out_d[w, nt * 98:(nt + 1) * 98, :],
                                      y_sb[:, :])

            for w in range(WPC):
                # ---- input DMAs ----
                xa = xa_pool.tile([97, 392], BF16, tag="xa")
                nc.sync.dma_start(xa[:, :], xa_d[w])
                ebm = ebm_pool.tile([98, 6 * 392], BF16, tag="ebm")
                nc.sync.dma_start(ebm[:, :], ebm_d[w])

                # ---- qk projection: 4 feature tiles of 96 ----
                # ft0=q h0-2, ft1=q h3-5, ft2=k h0-2, ft3=k h3-5
                qk_sb = []
                for ft in range(4):
                    ps = p_psum.tile([128, 512], F32, tag="ps2")
                    for kt in range(2):
                        nc.tensor.matmul(
                            ps[0:96, 0:196],
                            wqk_t[kt][:, ft * 96:(ft + 1) * 96],
                            xa[0:96, kt * 196:(kt + 1) * 196],
                            start=(kt == 0), stop=(kt == 1),
                        )
                    t = qk_pool.tile([96, 196], BF16, tag=f"qk{ft}")
                    if ft % 2 == 0:
                        nc.vector.tensor_copy(t[:, :], ps[0:96, 0:196])
                    else:
                        nc.scalar.copy(t[:, :], ps[0:96, 0:196])
                    qk_sb.append(t)
                qT_A, qT_B, kT_A, kT_B = qk_sb

                # ---- v projection (x stationary, ones-augmented) ----
                vaug = []
                for mt in range(2):
                    ps = p_psum.tile([128, 512], F32, tag="ps2")
                    for kt in range(2):
                        nc.tensor.matmul(
                            ps[0:98, 0:198],
                            xa[0:97, kt * 196 + mt * 98: kt * 196 + mt * 98 + 98],
                            wv_t[kt][:, :],
                            start=(kt == 0), stop=(kt == 1),
                        )
                    va = vaug_pool.tile([98, 198], BF16, tag="vaug")
                    nc.vector.tensor_copy(va[:, :], ps[0:98, 0:198])
                    vaug.append(va)

                # ---- per half (3 heads): QK^T -> S^T -> exp -> mul -> PV ----
                # half 0 = heads 0-2 (A tiles), half 1 = heads 3-5 (B tiles)
                s_sb = rr_pool.tile([1, 1176], F32, tag="ssb")
                U_halves = []
                P = ep_pool.tile([98, 6 * 392], BF16, tag="P")
                O_halves = []
                for hf in range(2):
                    kTh, qTh = (kT_A, qT_A) if hf == 0 else (kT_B, qT_B)
                    S = s_psum.tile([98, 3 * 512], F32, tag="S")
                    for hl in range(3):
                        for mt in range(2):
                            nc.tensor.matmul(
                                S[:, hl * 512 + mt * 196: hl * 512 + (mt + 1) * 196],
                                kTh[32 * hl:32 * hl + 32, mt * 98: mt * 98 + 98],
                                qTh[32 * hl:32 * hl + 32, :],
                                start=True, stop=True,
                                tile_position=(32 * hl, 0),
                            )
                    # exp across the 3 banks in one instr
                    E = ep_pool.tile([98, 3 * 392], BF16, tag="E")
                    S3 = S[:, :].rearrange("p (h x) -> p h x", h=3)[:, :, 0:392]
                    E3 = E[:, :].rearrange("p (h x) -> p h x", h=3)
                    nc.scalar.activation(E3, S3, mybir.ActivationFunctionType.Exp)
                    nc.vector.tensor_mul(
                        P[:, hf * 1176:(hf + 1) * 1176], E[:, :],
                        ebm[:, hf * 1176:(hf + 1) * 1176])

                    # PV into one 1-bank tile:
                    # local0: rows0-32 free0:196; local1: rows64-96 free0:196;
                    # local2: rows0-32 free196:392
                    O = o_psum.tile([98, 512], F32, tag="O")
                    O_halves.append(O)
                    for hl in range(3):
                        h = 3 * hf + hl
                        row = 64 if hl == 1 else 0
                        fo = 196 if hl == 2 else 0
                        for mt in range(2):
                            nc.tensor.matmul(
                                O[row:row + 33, fo:fo + 196],
                                vaug[mt][:, 33 * h: 33 * h + 33],
                                P[:, h * 392 + mt * 196: h * 392 + (mt + 1) * 196],
                                start=(mt == 0), stop=(mt == 1),
                            )
                    # copy O (incl s rows) to SBUF to free PSUM quickly
                    U_e = zt_pool.tile([33, 392], BF16, tag="Ue")
                    U_o = zt_pool.tile([33, 196], BF16, tag="Uo")
                    if hf == 0:
                        nc.scalar.copy(U_e[:, :], O[0:33, 0:392])
                        nc.vector.tensor_copy(U_o[:, :], O[64:97, 0:196])
                    else:
                        nc.vector.tensor_copy(U_e[:, :], O[0:33, 0:392])
                        nc.scalar.copy(U_o[:, :], O[64:97, 0:196])
                    U_halves.append((U_e, U_o))
                    # s rows -> staging: [s_l0|s_l2] at U_e row32, s_l1 at U_o
                    nc.vector.tensor_copy(s_sb[0:1, hf * 588: hf * 588 + 392],
                                          U_e[32:33, :])
                    nc.vector.tensor_copy(s_sb[0:1, hf * 588 + 392:(hf + 1) * 588],
                                          U_o[32:33, :])

                # ---- reciprocal: SBUF->SBUF transpose, recip on 98 lanes,
                #      transpose back, partition-broadcast on gpsimd ----
                s_t = rr_pool.tile([98, 12], F32, tag="st")
                nc.gpsimd.dma_start(s_t[:, :], s_sb[0:1, :])
                r_t = rr_pool.tile([98, 12], BF16, tag="rt")
                with nc.allow_low_precision(reason="softmax recip; rel_err gate 2e-2"):
                    nc.vector.reciprocal(r_t[:, :], s_t[:, :])
                r_row = rr_pool.tile([1, 1176], BF16, tag="rrow")
                nc.gpsimd.dma_start(r_row[0:1, :], r_t[:, :])
                R_all = rr_pool.tile([32, 1176], BF16, tag="Rall")
                nc.gpsimd.partition_broadcast(R_all[:, :], r_row[0:1, :])

                prev = carried.get("st")
                carried["st"] = {"U": U_halves, "R": R_all, "w": w}
                if prev is not None:
                    back(prev)
            back(carried["st"])
    nc.compile()
    return nc


def _host_precompute(x, w_qkv, w_proj, bias_table, mask, rel_index):
    scale = HD ** (-0.5)
    wq = np.array(w_qkv, np.float32).copy()
    wq[0:C] *= scale  # fold softmax scale into q weights

    # xa[w, p, kt*196 + j] = x[w, j, kt*96 + p]; row 96: kt0->0, kt1->1
    xT = np.ascontiguousarray(np.transpose(np.asarray(x, np.float32), (0, 2, 1)))
    xa = np.zeros((B, 97, 392), np.float32)
    xa[:, 0:96, 0:196] = xT[:, 0:96]
    xa[:, 0:96, 196:392] = xT[:, 96:192]
    xa[:, 96, 196:392] = 1.0

    # wqk[kt, p, f] = wq[f, kt*96+p]  (f < 384: q then k features)
    wqkT = wq[0:384].T  # [192, 384]
    wqk = np.stack([wqkT[0:96], wqkT[96:192]])

    # wv[kt, p, 33h+d] = wq[384+32h+d, kt*96+p]; ones row kt1 p=96
    wv = np.zeros((2, 97, 198), np.float32)
    wvT = wq[384:576].T  # [192, 192] [c, (h,d)]
    for h in range(H):
        wv[0, 0:96, 33 * h: 33 * h + 32] = wvT[0:96, 32 * h: 32 * h + 32]
        wv[1, 0:96, 33 * h: 33 * h + 32] = wvT[96:192, 32 * h: 32 * h + 32]
        wv[1, 96, 33 * h + 32] = 1.0

    # wp[kt, p, c'] = w_proj[c', head_perm...] matching zt row order
    wp = np.zeros((2, 96, 192), np.float32)
    wpT = np.asarray(w_proj, np.float32).T  # [c, c']
    head_rows = {0: (0, 0), 2: (0, 64), 1: (0, 32),
                 3: (1, 0), 5: (1, 64), 4: (1, 32)}
    for h, (kt, row) in head_rows.items():
        wp[kt, row:row + 32, :] = wpT[32 * h: 32 * h + 32]

    # EBM[w, p, h*392 + mt*196 + n] = exp(bias[n, m, h] + mask[w, n, m]),
    # m = mt*98 + p
    bias = np.asarray(bias_table, np.float32)[np.asarray(rel_index).reshape(-1)]
    bias = bias.reshape(N, N, H)  # [n, m, h]
    biasT = np.transpose(bias, (2, 1, 0))  # [h, m, n]
    maskT = np.transpose(np.asarray(mask, np.float32), (0, 2, 1))  # [g, m, n]
    ebm = np.exp(biasT[None] + maskT[:, None])  # [g, h, m, n]
    ebm = ebm.reshape(NG, H, 2, MT, N).transpose(0, 3, 1, 2, 4)
    ebm = np.ascontiguousarray(ebm.reshape(NG, MT, H * 392))

    return (xa.astype(NPBF16), wqk.astype(NPBF16), wv.astype(NPBF16),
            wp.astype(NPBF16), ebm.astype(NPBF16))


def kernel(x, w_qkv, w_proj, b_proj, bias_table, mask, rel_index):
    xa, wqk, wv, wp, ebm = _host_precompute(
        x, w_qkv, w_proj, bias_table, mask, rel_index)

    if "nc" not in _CACHE:
        _CACHE["nc"] = _build_nc()
    nc = _CACHE["nc"]

    in_maps = []
    for c in range(NCORES):
        in_maps.append({
            "xa": np.ascontiguousarray(xa[c * WPC:(c + 1) * WPC]),
            "ebm": ebm,  # window w on core uses mask (64c+w) % 64 = w
            "wqk": wqk, "wv": wv, "wp": wp,
        })

    res = bass_utils.run_bass_kernel_spmd(nc, in_maps, core_ids=list(range(NCORES)))
    out = np.concatenate([res.results[c]["out"] for c in range(NCORES)], axis=0)
    out = out.astype(np.float32) + np.asarray(b_proj, np.float32)[None, None, :]
    return out
